# revision 1
# baseline (speedup 1.0000x reference)
"""GATv3Conv Trainium2 kernel (8 NeuronCores, SPMD).

Strategy (v2):
  - Shard EDGES by destination-node slice (core k owns dst in [k*6250,(k+1)*6250)).
    Segment softmax + aggregation are fully core-local (no collectives).
  - Each core redundantly computes LayerNorm + the src/val GEMMs for ALL nodes
    into a bf16 table [N,512] = [fs|fv] in its HBM (features (f,h)-major so the
    per-edge exp-broadcast multiply hits the DVE 2x 16-bit path), and the dst
    GEMM only for its own slice (kept in SBUF as bf16).
  - Edge phase, per 128-dst-node window, edges in 128-edge chunks:
      * fsv rows gathered via gpsimd.dma_gather (1 call per table-half).
      * evT[f,e] = (one-hot dst gather of fd via matmul) + (fs^T via
        identity-moving matmul), accumulated in PSUM.
      * silu on ACT from PSUM (2-chunk groups).
      * score[e,h] via PE: lhsT=sevT half, rhs=block-diag attn [128,8] - the
        8-wide output makes these matmuls nearly free.
      * one Exp per window (scores [P,CPW*8]); windows processed in pairs with
        silu/exp lag so ACT table loads halve.
      * md = fv * e8 (DVE bf16 2x, (f,h)-major broadcast), aggregation +
        denominators via one-hot matmuls into a [P,264] PSUM accumulator.
  - Softmax division deferred to the end: out = silu(num/den + h). exp() uses
    raw scores (no segment max): scores are O(+-10); identical to reference.
"""

import numpy as np

N_NODES = 50000
IN_FEATS = 256
OUT_FEATS = 256
NUM_HEADS = 8
FPH = OUT_FEATS // NUM_HEADS   # 32
LN_EPS = 1e-5
N_CORES = 8
SLICE = N_NODES // N_CORES     # 6250
P = 128
NWIN = (SLICE + P - 1) // P    # 49
SLICE_PAD = NWIN * P           # 6272
N_PAD = ((N_NODES + P - 1) // P) * P   # 50048
NTILES = N_PAD // P            # 391
HALF = 32768
TBL_COLS = 2 * OUT_FEATS       # 512
AGG_COLS = OUT_FEATS + NUM_HEADS  # 264
ATILE = 4                      # node tiles per phase-A DMA batch

# new feature order is (f, h)-major: new col j=f*8+h <- old col h*32+f
_OLD_OF_NEW = (np.arange(OUT_FEATS) % NUM_HEADS) * FPH + \
    np.arange(OUT_FEATS) // NUM_HEADS

_CACHE = {}


def _build_nc(lowC, highC, reps=1, phases="ABC"):
    import concourse.bacc as bacc
    import concourse.tile as tile
    from concourse import mybir
    from contextlib import ExitStack

    f32 = mybir.dt.float32
    bf16 = mybir.dt.bfloat16
    i16 = mybir.dt.int16
    AF = mybir.ActivationFunctionType
    Alu = mybir.AluOpType

    lowC = list(lowC)
    highC = list(highC)
    cpw = [l + h for l, h in zip(lowC, highC)]
    cbase = np.concatenate([[0], np.cumsum(cpw)]).astype(int)
    C_TOT = int(cbase[-1])
    CPWMX = max(cpw)
    LCMX = max(lowC)
    HCMX = max(max(highC), 1)
    icols = 8 * C_TOT  # int16 idx cols (128 idx -> 8 cols of 16)

    nc = bacc.Bacc(None, target_bir_lowering=False)

    featb_t = nc.dram_tensor("featb", [N_PAD, IN_FEATS], bf16, kind="ExternalInput")
    featmy_t = nc.dram_tensor("featmy", [SLICE_PAD, IN_FEATS], bf16,
                              kind="ExternalInput")
    wfsv_t = nc.dram_tensor("wfsv", [P, 2, TBL_COLS], bf16, kind="ExternalInput")
    wfd_t = nc.dram_tensor("wfd", [P, 2, OUT_FEATS], bf16, kind="ExternalInput")
    attnb_t = nc.dram_tensor("attnb", [P, 2, NUM_HEADS], bf16, kind="ExternalInput")
    identb_t = nc.dram_tensor("identb", [P, P], bf16, kind="ExternalInput")
    iotab_t = nc.dram_tensor("iotab", [P, P], bf16, kind="ExternalInput")
    dstf_t = nc.dram_tensor("dstf", [P, C_TOT], f32, kind="ExternalInput")
    gidx_t = nc.dram_tensor("gidx", [P, icols], i16, kind="ExternalInput")
    ohg_t = nc.dram_tensor("ohg", [P, C_TOT, P], bf16, kind="ExternalInput")
    out_t = nc.dram_tensor("outmy", [SLICE_PAD, OUT_FEATS], f32,
                           kind="ExternalOutput")

    fsv_t = nc.dram_tensor("fsvtbl", [N_PAD, TBL_COLS], bf16, kind="Internal")
    hupd_t = nc.dram_tensor("hupdtbl", [NWIN, P, AGG_COLS], bf16, kind="Internal")

    with tile.TileContext(nc) as tc, ExitStack() as ctx:
        if reps > 1:
            ctx.enter_context(tc.For_i(0, reps, 1))
        const = ctx.enter_context(tc.tile_pool(name="const", bufs=1))
        persist = ctx.enter_context(tc.tile_pool(name="persist", bufs=1))

        wfsv = const.tile([P, 2, TBL_COLS], bf16)
        nc.sync.dma_start(out=wfsv, in_=wfsv_t[:, :, :])
        wfd = const.tile([P, 2, OUT_FEATS], bf16)
        nc.sync.dma_start(out=wfd, in_=wfd_t[:, :, :])
        attnb = const.tile([P, 2, NUM_HEADS], bf16)
        nc.sync.dma_start(out=attnb, in_=attnb_t[:, :, :])
        identb = const.tile([P, P], bf16)
        nc.sync.dma_start(out=identb, in_=identb_t[:, :])
        iotab = const.tile([P, P], bf16)
        nc.sync.dma_start(out=iotab, in_=iotab_t[:, :])
        dstf = const.tile([P, C_TOT], f32)
        nc.sync.dma_start(out=dstf, in_=dstf_t[:, :])
        gidx = const.tile([P, icols], i16)
        nc.sync.dma_start(out=gidx, in_=gidx_t[:, :])
        eps_c = const.tile([P, 1], f32)
        nc.vector.memset(eps_c[:], LN_EPS)

        fd_slice = persist.tile([P, NWIN, OUT_FEATS], bf16)
        stats_my = persist.tile([P, NWIN, 2], f32)   # (mean, rstd)

        # ---------------- Phase A: LN + GEMM tables ----------------
        def node_sweep(src_dram, ntiles, emit, wtile, wcols):
            with tc.tile_pool(name="a_sb", bufs=3) as sbp, \
                 tc.tile_pool(name="a_ps", bufs=2, space="PSUM") as psp:
                for t0 in range(0, ntiles, ATILE):
                    bt = min(ATILE, ntiles - t0)
                    F4 = sbp.tile([P, ATILE, IN_FEATS], bf16, tag="F4")
                    nc.sync.dma_start(
                        out=F4[:, 0:bt, :],
                        in_=src_dram[t0 * P:(t0 + bt) * P, :].rearrange(
                            "(t p) f -> p t f", p=P))
                    mv4 = sbp.tile([P, ATILE, 2], f32, tag="mv4")
                    for i in range(bt):
                        st = sbp.tile([P, 6], f32, tag="st")
                        nc.vector.bn_stats(out=st[:], in_=F4[:, i, :])
                        nc.vector.bn_aggr(out=mv4[:, i, :], in_=st[:])
                    sd4 = sbp.tile([P, ATILE], f32, tag="sd4")
                    nc.scalar.activation(out=sd4[:, 0:bt], in_=mv4[:, 0:bt, 1],
                                         func=AF.Sqrt, bias=eps_c[:])
                    rstd4 = sbp.tile([P, ATILE], f32, tag="rstd4")
                    nc.vector.reciprocal(out=rstd4[:, 0:bt], in_=sd4[:, 0:bt])
                    h4 = sbp.tile([P, ATILE, IN_FEATS], bf16, tag="h4")
                    for i in range(bt):
                        nc.vector.tensor_scalar(
                            out=h4[:, i, :], in0=F4[:, i, :],
                            scalar1=mv4[:, i, 0:1], scalar2=rstd4[:, i:i + 1],
                            op0=Alu.subtract, op1=Alu.mult)
                    hT4 = sbp.tile([P, ATILE, 2, P], bf16, tag="hT4")
                    nc.sync.dma_start_transpose(out=hT4[:, 0:bt, :, :],
                                                in_=h4[:, 0:bt, :])
                    batch = []
                    for i in range(bt):
                        g = psp.tile([P, wcols], f32, tag="gemm")
                        nc.tensor.matmul(out=g[:], lhsT=hT4[:, i, 0, :],
                                         rhs=wtile[:, 0, :], start=True, stop=False)
                        nc.tensor.matmul(out=g[:], lhsT=hT4[:, i, 1, :],
                                         rhs=wtile[:, 1, :], start=False, stop=True)
                        emit(t0 + i, i, g, mv4, rstd4, sbp, batch)
                    if batch:
                        st4, bt0 = batch[0]
                        nc.sync.dma_start(
                            out=fsv_t[bt0 * P:(bt0 + bt) * P, :].rearrange(
                                "(t p) f -> p t f", p=P),
                            in_=st4[:, 0:bt, :])

        def emit_fsv(t, i, g, mv4, rstd4, sbp, batch):
            if i == 0:
                st4 = sbp.tile([P, ATILE, TBL_COLS], bf16, tag="fsv4")
                batch.append((st4, t))
            st4, _ = batch[0]
            nc.vector.tensor_copy(out=st4[:, i, 0:OUT_FEATS],
                                  in_=g[:, 0:OUT_FEATS])
            nc.scalar.activation(out=st4[:, i, OUT_FEATS:], in_=g[:, OUT_FEATS:],
                                 func=AF.Copy)

        def emit_fd(t, i, g, mv4, rstd4, sbp, batch):
            nc.vector.tensor_copy(out=fd_slice[:, t, 0:P], in_=g[:, 0:P])
            nc.scalar.activation(out=fd_slice[:, t, P:OUT_FEATS], in_=g[:, P:],
                                 func=AF.Copy)
            nc.vector.tensor_copy(out=stats_my[:, t, 0:1], in_=mv4[:, i, 0:1])
            nc.vector.tensor_copy(out=stats_my[:, t, 1:2],
                                  in_=rstd4[:, i:i + 1])

        node_sweep(featmy_t, NWIN, emit_fd, wfd, OUT_FEATS)
        if "A" in phases:
            node_sweep(featb_t, NTILES, emit_fsv, wfsv, TBL_COLS)

        # ---------------- Phase B: edge phase ----------------
        fsv_hi = fsv_t[HALF:N_PAD, :]
        if "B" not in phases:
            NWIN_B = 0
        else:
            NWIN_B = NWIN
        import os as _os
        _bb = lambda k, d: int(_os.environ.get(k, d))
        with tc.tile_pool(name="b_glo", bufs=_bb("BGL", 3)) as glop, \
             tc.tile_pool(name="b_ghi", bufs=_bb("BGH", 3)) as ghip, \
             tc.tile_pool(name="b_ohg", bufs=_bb("BOG", 3)) as ohgp, \
             tc.tile_pool(name="b_oha", bufs=_bb("BOA", 3)) as ohap, \
             tc.tile_pool(name="b_sev", bufs=_bb("BSV", 4)) as sevp, \
             tc.tile_pool(name="b_e8", bufs=_bb("BE8", 4)) as e8p, \
             tc.tile_pool(name="b_md", bufs=_bb("BMD", 3)) as mdp, \
             tc.tile_pool(name="b_hw", bufs=_bb("BHW", 3)) as hwp, \
             tc.tile_pool(name="b_ev", bufs=_bb("BEV", 3), space="PSUM") as evp, \
             tc.tile_pool(name="b_sc", bufs=_bb("BSC", 3), space="PSUM") as scp, \
             tc.tile_pool(name="b_hu", bufs=_bb("BHU", 2), space="PSUM") as hup:

            icol_of = np.concatenate([[0], np.cumsum([8 * c for c in cpw])])

            def win_gather(w):
                """Issue gathers + oh loads + one-hot builds for window w."""
                cb = cbase[w]
                icol = int(icol_of[w])
                tiles = {}
                ohg_w = ohgp.tile([P, CPWMX, P], bf16, tag="ohg")
                if LVL >= 2:
                    nc.sync.dma_start(out=ohg_w[:, 0:cpw[w], :],
                                      in_=ohg_t[:, cb:cb + cpw[w], :])
                import os as _os2
                _SEQDMA = _os2.environ.get("DBG_SEQDMA") == "1"
                MAXC = 8  # >1024 idx per dma_gather call wedges the device
                if lowC[w] and _SEQDMA:
                    G = glop.tile([P, LCMX, TBL_COLS], bf16, tag="Glo")
                    nc.sync.dma_start(
                        out=G[:, 0:lowC[w], :],
                        in_=fsv_t[0:lowC[w] * P, :].rearrange(
                            "(t p) f -> p t f", p=P))
                    tiles["lo"] = G
                    icol += 8 * lowC[w]
                if highC[w] and _SEQDMA:
                    G = ghip.tile([P, HCMX, TBL_COLS], bf16, tag="Ghi")
                    nc.sync.dma_start(
                        out=G[:, 0:highC[w], :],
                        in_=fsv_t[0:highC[w] * P, :].rearrange(
                            "(t p) f -> p t f", p=P))
                    tiles["hi"] = G
                    icol += 8 * highC[w]
                if lowC[w] and not _SEQDMA:
                    G = glop.tile([P, LCMX, TBL_COLS], bf16, tag="Glo")
                    o = 0
                    while o < lowC[w]:
                        g = min(MAXC, lowC[w] - o)
                        ni = g * P
                        nc.gpsimd.dma_gather(
                            out_ap=G[:, o:o + g, :], in_ap=fsv_t[:, :],
                            idxs_ap=gidx[:, icol:icol + ni // 16],
                            num_idxs=ni, num_idxs_reg=ni, elem_size=TBL_COLS)
                        icol += ni // 16
                        o += g
                    tiles["lo"] = G
                if highC[w] and not _SEQDMA:
                    G = ghip.tile([P, HCMX, TBL_COLS], bf16, tag="Ghi")
                    o = 0
                    while o < highC[w]:
                        g = min(MAXC, highC[w] - o)
                        ni = g * P
                        nc.gpsimd.dma_gather(
                            out_ap=G[:, o:o + g, :], in_ap=fsv_hi,
                            idxs_ap=gidx[:, icol:icol + ni // 16],
                            num_idxs=ni, num_idxs_reg=ni, elem_size=TBL_COLS)
                        icol += ni // 16
                        o += g
                    tiles["hi"] = G
                oha_w = ohap.tile([P, CPWMX, P], bf16, tag="oha")
                if LVL >= 2:
                    for c in range(cpw[w]):
                        nc.vector.tensor_scalar(
                            out=oha_w[:, c, :], in0=iotab[:],
                            scalar1=dstf[:, cb + c:cb + c + 1], scalar2=None,
                            op0=Alu.is_equal)
                return tiles, ohg_w, oha_w

            def chunk_of(w, c):
                """(section G-key, local idx) for chunk c of window w."""
                if c < lowC[w]:
                    return "lo", c
                return "hi", c - lowC[w]

            SG = int(_os.environ.get("BSG", "2"))  # chunks per silu group

            def score_group(w, tiles, ohg_w, score_ps, c0):
                gn = min(SG, cpw[w] - c0)
                ev2 = evp.tile([P, SG, 2, P], f32, tag="ev2")
                for i in range(gn if LVL >= 3 else 0):
                    sec, lc = chunk_of(w, c0 + i)
                    G = tiles[sec]
                    for b in range(2):
                        nc.tensor.matmul(
                            out=ev2[:, i, b, :],
                            lhsT=fd_slice[:, w, b * P:(b + 1) * P],
                            rhs=ohg_w[:, c0 + i, :], start=True, stop=False)
                        nc.tensor.matmul(
                            out=ev2[:, i, b, :],
                            lhsT=G[:, lc, b * P:(b + 1) * P],
                            rhs=identb[:], start=False, stop=True)
                sevT = sevp.tile([P, SG, 2, P], bf16, tag="sevT")
                if LVL >= 4:
                    nc.scalar.activation(out=sevT[:, 0:gn, :, :],
                                         in_=ev2[:, 0:gn, :, :], func=AF.Silu)
                for i in range(gn if LVL >= 5 else 0):
                    for b in range(2):
                        nc.tensor.matmul(
                            out=score_ps[:, c0 + i, :],
                            lhsT=sevT[:, i, b, :], rhs=attnb[:, b, :],
                            start=(b == 0), stop=(b == 1))

            def win_finish(w, tiles, ohg_w, oha_w, score_ps):
                e8_w = e8p.tile([P, CPWMX, NUM_HEADS], bf16, tag="e8")
                if LVL < 5:
                    if LVL >= 4.5 or True:
                        pass
                else:
                    nc.scalar.activation(out=e8_w[:, 0:cpw[w], :],
                                         in_=score_ps[:, 0:cpw[w], :], func=AF.Exp)
                mds = {}
                for sec, g, off in (("lo", lowC[w], 0), ("hi", highC[w], lowC[w])):
                    if not g or LVL < 6:
                        continue
                    G = tiles[sec]
                    md = mdp.tile([P, LCMX if sec == "lo" else HCMX, OUT_FEATS],
                                  bf16, tag="md" + sec)
                    nc.vector.tensor_tensor(
                        out=md[:, 0:g, :].rearrange("p c (f h) -> p c f h",
                                                    h=NUM_HEADS),
                        in0=G[:, 0:g, OUT_FEATS:].rearrange(
                            "p c (f h) -> p c f h", h=NUM_HEADS),
                        in1=e8_w[:, off:off + g, None, :].to_broadcast(
                            [P, g, FPH, NUM_HEADS]),
                        op=Alu.mult)
                    mds[sec] = md
                # note: matmul accumulation groups must stay contiguous per
                # PSUM region on HW - interleaving two regions corrupts one.
                hw_s = hwp.tile([P, AGG_COLS], bf16, tag="hw_s")
                if LVL < 7:
                    return
                if _os.environ.get("DBG_NOHUPD") == "1":
                    nc.vector.memset(hw_s[:], 0.5)
                else:
                    hupd = hup.tile([P, AGG_COLS], f32, tag="hupd")
                    for c in range(cpw[w]):
                        sec, lc = chunk_of(w, c)
                        nc.tensor.matmul(out=hupd[:, 0:OUT_FEATS],
                                         lhsT=oha_w[:, c, :], rhs=mds[sec][:, lc, :],
                                         start=(c == 0), stop=(c == cpw[w] - 1))
                    for c in range(cpw[w]):
                        nc.tensor.matmul(out=hupd[:, OUT_FEATS:],
                                         lhsT=oha_w[:, c, :], rhs=e8_w[:, c, :],
                                         start=(c == 0), stop=(c == cpw[w] - 1))
                    nc.vector.tensor_copy(out=hw_s[:], in_=hupd[:])
                nc.sync.dma_start(out=hupd_t[w], in_=hw_s[:])

            NPAR = int(_os.environ.get("BNP", "2"))  # windows in flight
            LVL = int(_os.environ.get("DBG_LEVEL", "99"))
            w = 0
            while w < NWIN_B:
                grp = list(range(w, min(w + NPAR, NWIN_B)))
                state = []
                for ww in grp:
                    tiles, ohg_w, oha_w = win_gather(ww)
                    score_ps = scp.tile([P, CPWMX, NUM_HEADS], f32, tag="score")
                    state.append((ww, tiles, ohg_w, oha_w, score_ps))
                maxg = max((cpw[ww] + SG - 1) // SG for ww in grp)
                for gi in range(maxg):
                    for (ww, tiles, ohg_w, oha_w, score_ps) in state:
                        if gi * SG < cpw[ww]:
                            score_group(ww, tiles, ohg_w, score_ps, gi * SG)
                for (ww, tiles, ohg_w, oha_w, score_ps) in state:
                    win_finish(ww, tiles, ohg_w, oha_w, score_ps)
                w += len(grp)

        # ---------------- Phase C: normalize + residual + silu ----------------
        with tc.tile_pool(name="c_sb", bufs=3) as sbp:
            for w0 in range(0, NWIN if "C" in phases else 0, ATILE):
                bt = min(ATILE, NWIN - w0)
                hu4 = sbp.tile([P, ATILE, AGG_COLS], bf16, tag="hu4")
                nc.sync.dma_start(out=hu4[:, 0:bt, :],
                                  in_=hupd_t[w0:w0 + bt].rearrange(
                                      "w p c -> p w c"))
                F4 = sbp.tile([P, ATILE, IN_FEATS], bf16, tag="F4")
                nc.sync.dma_start(
                    out=F4[:, 0:bt, :],
                    in_=featmy_t[w0 * P:(w0 + bt) * P, :].rearrange(
                        "(t p) f -> p t f", p=P))
                for i in range(bt):
                    w = w0 + i
                    h = sbp.tile([P, IN_FEATS], bf16, tag="h")
                    nc.vector.tensor_scalar(
                        out=h[:], in0=F4[:, i, :], scalar1=stats_my[:, w, 0:1],
                        scalar2=stats_my[:, w, 1:2],
                        op0=Alu.subtract, op1=Alu.mult)
                    den = sbp.tile([P, NUM_HEADS], f32, tag="den")
                    nc.vector.tensor_scalar_add(out=den[:],
                                                in0=hu4[:, i, OUT_FEATS:],
                                                scalar1=1e-30)
                    denr = sbp.tile([P, NUM_HEADS], f32, tag="denr")
                    nc.vector.reciprocal(out=denr[:], in_=den[:])
                    o = sbp.tile([P, FPH, NUM_HEADS], f32, tag="o")
                    nc.vector.tensor_tensor(
                        out=o[:],
                        in0=hu4[:, i, 0:OUT_FEATS].rearrange(
                            "p (f h) -> p f h", h=NUM_HEADS),
                        in1=denr[:, None, :].to_broadcast([P, FPH, NUM_HEADS]),
                        op=Alu.mult)
                    on = sbp.tile([P, OUT_FEATS], f32, tag="on")
                    nc.vector.tensor_tensor(
                        out=on[:].rearrange("p (h f) -> p h f", h=NUM_HEADS),
                        in0=o[:].rearrange("p f h -> p h f"),
                        in1=h[:].rearrange("p (h f) -> p h f", h=NUM_HEADS),
                        op=Alu.add)
                    oo = sbp.tile([P, OUT_FEATS], f32, tag="oo")
                    nc.scalar.activation(out=oo[:], in_=on[:], func=AF.Silu)
                    nc.sync.dma_start(out=out_t[w * P:(w + 1) * P, :], in_=oo[:])

    nc.compile()
    return nc, (lowC, highC)


def _derive_schedule(src, dst):
    """Per-window chunk counts (low/high table half), maxed over cores."""
    lowC = np.zeros(NWIN, np.int64)
    highC = np.zeros(NWIN, np.int64)
    for core in range(N_CORES):
        lo, hi = core * SLICE, (core + 1) * SLICE
        m = (dst >= lo) & (dst < hi)
        w_of = (dst[m] - lo) // P
        is_lo = src[m] < HALF
        cl = np.bincount(w_of[is_lo], minlength=NWIN)
        ch = np.bincount(w_of[~is_lo], minlength=NWIN)
        lowC = np.maximum(lowC, (cl + P - 1) // P)
        highC = np.maximum(highC, (ch + P - 1) // P)
    return tuple(int(x) for x in lowC), tuple(int(x) for x in highC)


def _prepare_core_inputs(core, src, dst, lowC, highC):
    import ml_dtypes
    cpw = [l + h for l, h in zip(lowC, highC)]
    C_TOT = sum(cpw)
    cbase = np.concatenate([[0], np.cumsum(cpw)]).astype(int)

    lo, hi = core * SLICE, (core + 1) * SLICE
    m = (dst >= lo) & (dst < hi)
    dsl = dst[m] - lo
    ssl = src[m]
    w_of = dsl // P
    is_lo = ssl < HALF

    slot_src = np.zeros((C_TOT, P), np.int64)
    slot_doff = np.full((C_TOT, P), -1.0, np.float32)

    # sort each (window, table-half) section by src so the gather reads the
    # table in ascending row order (HBM locality)
    order = np.lexsort((ssl, ~is_lo, w_of))
    ssl_o, dsl_o, w_o, lo_o = ssl[order], dsl[order], w_of[order], is_lo[order]
    for w in range(NWIN):
        wm = w_o == w
        for half, cb, g in ((True, cbase[w], lowC[w]),
                            (False, cbase[w] + lowC[w], highC[w])):
            sel = wm & (lo_o == half)
            s_w = ssl_o[sel]
            d_w = dsl_o[sel] - w * P
            n = len(s_w)
            assert n <= g * P, (core, w, half, n, g * P)
            flat_s = slot_src[cb:cb + g].reshape(-1)
            flat_d = slot_doff[cb:cb + g].reshape(-1)
            flat_s[:n] = s_w
            flat_d[:n] = d_w

    dstf = slot_doff.T.copy()  # [P, C_TOT] f32

    ohg = np.zeros((P, C_TOT, P), ml_dtypes.bfloat16)
    cc, ee = np.nonzero(slot_doff >= 0)
    ohg[slot_doff[cc, ee].astype(np.int64), cc, ee] = 1

    idx_cols = []
    for w in range(NWIN):
        for half, cb, g in ((True, cbase[w], lowC[w]),
                            (False, cbase[w] + lowC[w], highC[w])):
            if not g:
                continue
            s = slot_src[cb:cb + g].reshape(-1).copy()
            if not half:
                s = np.maximum(s - HALF, 0)
            idx = s.astype(np.int16)
            idx_cols.append(np.tile(idx.reshape(-1, 16).T, (8, 1)))
    gidx = np.concatenate(idx_cols, axis=1).astype(np.int16)
    return dstf, gidx, ohg


def _shared_inputs(inputs):
    import ml_dtypes
    feat = np.asarray(inputs["feat"], np.float32)
    Wsrc = np.asarray(inputs["Wsrc"], np.float32)
    Wdst = np.asarray(inputs["Wdst"], np.float32)
    Wval = np.asarray(inputs["Wval"], np.float32)
    attn = np.asarray(inputs["attn"], np.float32).reshape(NUM_HEADS, FPH)

    featb = np.zeros((N_PAD, IN_FEATS), ml_dtypes.bfloat16)
    featb[:N_NODES] = feat.astype(ml_dtypes.bfloat16)

    # weights transposed + output-column permuted to (f,h)-major
    WsrcP = Wsrc[_OLD_OF_NEW, :]   # [256 newcol, 256 in]
    WvalP = Wval[_OLD_OF_NEW, :]
    WdstP = Wdst[_OLD_OF_NEW, :]
    wfsv = np.zeros((P, 2, TBL_COLS), np.float32)
    for b in range(2):
        wfsv[:, b, 0:OUT_FEATS] = WsrcP[:, b * P:(b + 1) * P].T
        wfsv[:, b, OUT_FEATS:] = WvalP[:, b * P:(b + 1) * P].T
    wfd = np.zeros((P, 2, OUT_FEATS), np.float32)
    for b in range(2):
        wfd[:, b, :] = WdstP[:, b * P:(b + 1) * P].T

    attnb = np.zeros((P, 2, NUM_HEADS), np.float32)
    for b in range(2):
        j = b * P + np.arange(P)
        attnb[np.arange(P), b, j % NUM_HEADS] = attn[j % NUM_HEADS,
                                                     j // NUM_HEADS]

    identb = np.eye(P, dtype=ml_dtypes.bfloat16)
    iotab = np.tile(np.arange(P, dtype=ml_dtypes.bfloat16).reshape(1, P),
                    (P, 1))
    bf = ml_dtypes.bfloat16
    return (featb, feat, wfsv.astype(bf), wfd.astype(bf), attnb.astype(bf),
            identb, iotab)


def make_in_maps(inputs, lowC, highC):
    import ml_dtypes
    featb, feat, wfsv, wfd, attnb, identb, iotab = _shared_inputs(inputs)
    src = np.asarray(inputs["src"], np.int64)
    dst = np.asarray(inputs["dst"], np.int64)
    in_maps = []
    for core in range(N_CORES):
        dstf, gidx, ohg = _prepare_core_inputs(core, src, dst, lowC, highC)
        featmy = np.zeros((SLICE_PAD, IN_FEATS), ml_dtypes.bfloat16)
        n = min(SLICE_PAD, N_NODES - core * SLICE)
        featmy[:n] = feat[core * SLICE:core * SLICE + n].astype(
            ml_dtypes.bfloat16)
        in_maps.append(dict(
            featb=featb, featmy=featmy, wfsv=wfsv, wfd=wfd, attnb=attnb,
            identb=identb, iotab=iotab, dstf=dstf, gidx=gidx, ohg=ohg,
        ))
    return in_maps


def kernel(**inputs):
    import concourse.bass_utils as bass_utils

    for b in ("bsrc", "bdst", "bval"):
        assert not np.any(np.asarray(inputs[b])), \
            "nonzero biases unsupported by this kernel"
    src = np.asarray(inputs["src"], np.int64)
    dst = np.asarray(inputs["dst"], np.int64)

    lowC, highC = _derive_schedule(src, dst)
    key = (lowC, highC)
    if key not in _CACHE:
        _CACHE[key] = _build_nc(lowC, highC)
    nc, _ = _CACHE[key]

    in_maps = make_in_maps(inputs, lowC, highC)
    res = bass_utils.run_bass_kernel_spmd(nc, in_maps, core_ids=list(range(N_CORES)))
    out = np.concatenate(
        [res.results[c]["outmy"][:SLICE] for c in range(N_CORES)], axis=0)
    return np.ascontiguousarray(out.astype(np.float32))



# revision 34
# speedup vs baseline: 1.5896x; 1.5896x over previous
"""GATv3Conv Trainium2 kernel (8 NeuronCores, SPMD).

Strategy (v2):
  - Shard EDGES by destination-node slice (core k owns dst in [k*6250,(k+1)*6250)).
    Segment softmax + aggregation are fully core-local (no collectives).
  - Each core redundantly computes LayerNorm + the src/val GEMMs for ALL nodes
    into a bf16 table [N,512] = [fs|fv] in its HBM (features (f,h)-major so the
    per-edge exp-broadcast multiply hits the DVE 2x 16-bit path), and the dst
    GEMM only for its own slice (kept in SBUF as bf16).
  - Edge phase, per 128-dst-node window, edges in 128-edge chunks:
      * fsv rows gathered via gpsimd.dma_gather (1 call per table-half).
      * evT[f,e] = (one-hot dst gather of fd via matmul) + (fs^T via
        identity-moving matmul), accumulated in PSUM.
      * silu on ACT from PSUM (2-chunk groups).
      * score[e,h] via PE: lhsT=sevT half, rhs=block-diag attn [128,8] - the
        8-wide output makes these matmuls nearly free.
      * one Exp per window (scores [P,CPW*8]); windows processed in pairs with
        silu/exp lag so ACT table loads halve.
      * md = fv * e8 (DVE bf16 2x, (f,h)-major broadcast), aggregation +
        denominators via one-hot matmuls into a [P,264] PSUM accumulator.
  - Softmax division deferred to the end: out = silu(num/den + h). exp() uses
    raw scores (no segment max): scores are O(+-10); identical to reference.
"""

import numpy as np

N_NODES = 50000
IN_FEATS = 256
OUT_FEATS = 256
NUM_HEADS = 8
FPH = OUT_FEATS // NUM_HEADS   # 32
LN_EPS = 1e-5
N_CORES = 8
SLICE = N_NODES // N_CORES     # 6250
P = 128
NWIN = (SLICE + P - 1) // P    # 49
SLICE_PAD = NWIN * P           # 6272
N_PAD = ((N_NODES + P - 1) // P) * P   # 50048
NTILES = N_PAD // P            # 391
HALF = 32768
TBL_COLS = 2 * OUT_FEATS       # 512
AGG_COLS = OUT_FEATS + NUM_HEADS  # 264
import os as _osm
ATILE = int(_osm.environ.get("ATILE", "4"))  # node tiles per phase-A DMA batch

# new feature order is (f, h)-major: new col j=f*8+h <- old col h*32+f
_OLD_OF_NEW = (np.arange(OUT_FEATS) % NUM_HEADS) * FPH + \
    np.arange(OUT_FEATS) // NUM_HEADS

_CACHE = {}


def _build_nc(lowC, highC, reps=1, phases="ABC"):
    import concourse.bacc as bacc
    import concourse.tile as tile
    from concourse import mybir
    from contextlib import ExitStack

    f32 = mybir.dt.float32
    bf16 = mybir.dt.bfloat16
    f8 = mybir.dt.float8e4
    i16 = mybir.dt.int16
    AF = mybir.ActivationFunctionType
    Alu = mybir.AluOpType

    lowC = list(lowC)
    highC = list(highC)
    cpw = [l + h for l, h in zip(lowC, highC)]
    cbase = np.concatenate([[0], np.cumsum(cpw)]).astype(int)
    C_TOT = int(cbase[-1])
    CPWMX = max(cpw)
    LCMX = max(lowC)
    HCMX = max(max(highC), 1)
    icols = 8 * C_TOT  # int16 idx cols (128 idx -> 8 cols of 16)

    import os as _os0
    GQN = int(_os0.environ.get("GQN", "4"))
    nc = bacc.Bacc(None, target_bir_lowering=False, num_swdge_queues=GQN)

    featTb_t = nc.dram_tensor("featTb", [NTILES, P, 2, P], bf16,
                              kind="ExternalInput")
    featmyT_t = nc.dram_tensor("featmyT", [NWIN, P, 2, P], bf16,
                               kind="ExternalInput")
    featmy_t = nc.dram_tensor("featmy", [SLICE_PAD, IN_FEATS], bf16,
                              kind="ExternalInput")
    wfsv_t = nc.dram_tensor("wfsv", [P, 2, TBL_COLS], bf16, kind="ExternalInput")
    wfd_t = nc.dram_tensor("wfd", [P, 2, OUT_FEATS], bf16, kind="ExternalInput")
    attnb_t = nc.dram_tensor("attnb", [P, 2, NUM_HEADS], bf16, kind="ExternalInput")
    identb_t = nc.dram_tensor("identb", [P, P], bf16, kind="ExternalInput")
    iotab_t = nc.dram_tensor("iotab", [P, P], bf16, kind="ExternalInput")
    dstf_t = nc.dram_tensor("dstf", [P, C_TOT], bf16, kind="ExternalInput")
    gidx_t = nc.dram_tensor("gidx", [P, icols], i16, kind="ExternalInput")
    ohg_t = nc.dram_tensor("ohg", [P, C_TOT, P], f8, kind="ExternalInput")
    out_t = nc.dram_tensor("outmy", [SLICE_PAD, OUT_FEATS], bf16,
                           kind="ExternalOutput")

    fsv_t = nc.dram_tensor("fsvtbl", [N_PAD, TBL_COLS], bf16, kind="Internal")
    hupd_t = nc.dram_tensor("hupdtbl", [NWIN, P, AGG_COLS], bf16, kind="Internal")

    with tile.TileContext(nc) as tc, ExitStack() as ctx:
        if reps > 1:
            ctx.enter_context(tc.For_i(0, reps, 1))
        const = ctx.enter_context(tc.tile_pool(name="const", bufs=1))
        persist = ctx.enter_context(tc.tile_pool(name="persist", bufs=1))

        wfsv = const.tile([P, 2, TBL_COLS], bf16)
        nc.sync.dma_start(out=wfsv, in_=wfsv_t[:, :, :])
        wfd = const.tile([P, 2, OUT_FEATS], bf16)
        nc.sync.dma_start(out=wfd, in_=wfd_t[:, :, :])
        attnb = const.tile([P, 2, NUM_HEADS], bf16)
        nc.sync.dma_start(out=attnb, in_=attnb_t[:, :, :])
        identb = const.tile([P, P], bf16)
        nc.sync.dma_start(out=identb, in_=identb_t[:, :])
        iotab = const.tile([P, P], bf16)
        nc.sync.dma_start(out=iotab, in_=iotab_t[:, :])
        dstf = const.tile([P, C_TOT], bf16)
        nc.sync.dma_start(out=dstf, in_=dstf_t[:, :])
        gidx = const.tile([P, icols], i16)
        nc.sync.dma_start(out=gidx, in_=gidx_t[:, :])
        eps_c = const.tile([P, 1], f32)
        nc.vector.memset(eps_c[:], LN_EPS)
        ones_c = const.tile([P, 1], bf16)
        nc.vector.memset(ones_c[:], 1.0)

        fd_slice = persist.tile([P, NWIN, OUT_FEATS], bf16)
        stats_my = persist.tile([P, NWIN, 2], f32)   # (mean, rstd)

        # ---------------- Phase A: LN + GEMM tables ----------------
        # feat arrives pre-transposed/tiled from host ([tile, f-part, blk, node]).
        # LN mean is folded into centered weights; stats (sum/sumsq) come from
        # ones-matmuls on hT and ACT Square, so no natural-layout load at all.
        def node_sweep(srcT_dram, ntiles, emit, wtile, wcols):
            import os as _osA
            with tc.tile_pool(name="a_sb", bufs=int(_osA.environ.get("ASB", "4"))) as sbp, \
                 tc.tile_pool(name="a_ps", bufs=int(_osA.environ.get("APS", "4")), space="PSUM") as psp, \
                 tc.tile_pool(name="a_st", bufs=int(_osA.environ.get("AST", "3")), space="PSUM") as stp:
                for t0 in range(0, ntiles, ATILE):
                    bt = min(ATILE, ntiles - t0)
                    hT4 = sbp.tile([P, ATILE, 2, P], bf16, tag="hT4")
                    nc.sync.dma_start(
                        out=hT4[:, 0:bt, :, :],
                        in_=srcT_dram[t0:t0 + bt].rearrange("t p b e -> p t b e"))
                    sq4 = sbp.tile([P, ATILE, 2, P], bf16, tag="sq4")
                    nc.scalar.activation(out=sq4[:, 0:bt], in_=hT4[:, 0:bt],
                                         func=AF.Square)
                    sps = stp.tile([P, ATILE, 2], f32, tag="sps")
                    for i in range(bt):
                        for b in range(2):
                            nc.tensor.matmul(out=sps[:, i, 0:1],
                                             lhsT=hT4[:, i, b, :], rhs=ones_c[:],
                                             start=(b == 0), stop=(b == 1))
                        for b in range(2):
                            nc.tensor.matmul(out=sps[:, i, 1:2],
                                             lhsT=sq4[:, i, b, :], rhs=ones_c[:],
                                             start=(b == 0), stop=(b == 1))
                    # var = sumsq/256 - (sum/256)^2 ; rstd = 1/sqrt(var+eps)
                    mean4 = sbp.tile([P, ATILE], f32, tag="mean4")
                    nc.vector.tensor_scalar(out=mean4[:, 0:bt],
                                            in0=sps[:, 0:bt, 0],
                                            scalar1=1.0 / IN_FEATS, scalar2=None,
                                            op0=Alu.mult)
                    mu2 = sbp.tile([P, ATILE], f32, tag="mu2")
                    nc.scalar.activation(out=mu2[:, 0:bt], in_=sps[:, 0:bt, 0],
                                         func=AF.Square, scale=1.0 / IN_FEATS)
                    var4 = sbp.tile([P, ATILE], f32, tag="var4")
                    nc.vector.tensor_scalar(out=var4[:, 0:bt],
                                            in0=sps[:, 0:bt, 1],
                                            scalar1=1.0 / IN_FEATS,
                                            scalar2=None, op0=Alu.mult)
                    vc4 = sbp.tile([P, ATILE], f32, tag="vc4")
                    nc.vector.tensor_tensor(out=vc4[:, 0:bt], in0=var4[:, 0:bt],
                                            in1=mu2[:, 0:bt], op=Alu.subtract)
                    sd4 = sbp.tile([P, ATILE], f32, tag="sd4")
                    nc.scalar.activation(out=sd4[:, 0:bt], in_=vc4[:, 0:bt],
                                         func=AF.Sqrt, bias=eps_c[:])
                    rstd4 = sbp.tile([P, ATILE], f32, tag="rstd4")
                    nc.vector.reciprocal(out=rstd4[:, 0:bt], in_=sd4[:, 0:bt])
                    batch = []
                    for i in range(bt):
                        g = psp.tile([P, wcols], f32, tag="gemm")
                        nc.tensor.matmul(out=g[:], lhsT=hT4[:, i, 0, :],
                                         rhs=wtile[:, 0, :], start=True, stop=False)
                        nc.tensor.matmul(out=g[:], lhsT=hT4[:, i, 1, :],
                                         rhs=wtile[:, 1, :], start=False, stop=True)
                        emit(t0 + i, i, g, mean4, rstd4, sbp, batch)
                    if batch:
                        st4, bt0 = batch[0]
                        nc.sync.dma_start(
                            out=fsv_t[bt0 * P:(bt0 + bt) * P, :].rearrange(
                                "(t p) f -> p t f", p=P),
                            in_=st4[:, 0:bt, :])

        def emit_fsv(t, i, g, mean4, rstd4, sbp, batch):
            if i == 0:
                st4 = sbp.tile([P, ATILE, TBL_COLS], bf16, tag="fsv4")
                batch.append((st4, t))
            st4, _ = batch[0]
            nc.vector.tensor_scalar(
                out=st4[:, i, 0:OUT_FEATS], in0=g[:, 0:OUT_FEATS],
                scalar1=rstd4[:, i:i + 1], scalar2=None, op0=Alu.mult)
            nc.scalar.activation(out=st4[:, i, OUT_FEATS:], in_=g[:, OUT_FEATS:],
                                 func=AF.Copy, scale=rstd4[:, i:i + 1])

        def emit_fd(t, i, g, mean4, rstd4, sbp, batch):
            nc.vector.tensor_scalar(
                out=fd_slice[:, t, 0:P], in0=g[:, 0:P],
                scalar1=rstd4[:, i:i + 1], scalar2=None, op0=Alu.mult)
            nc.scalar.activation(out=fd_slice[:, t, P:OUT_FEATS], in_=g[:, P:],
                                 func=AF.Copy, scale=rstd4[:, i:i + 1])
            nc.vector.tensor_copy(out=stats_my[:, t, 0:1], in_=mean4[:, i:i + 1])
            nc.vector.tensor_copy(out=stats_my[:, t, 1:2],
                                  in_=rstd4[:, i:i + 1])

        node_sweep(featmyT_t, NWIN, emit_fd, wfd, OUT_FEATS)
        if "A" in phases:
            node_sweep(featTb_t, NTILES, emit_fsv, wfsv, TBL_COLS)

        # ---------------- Phase B: edge phase ----------------
        fsv_hi = fsv_t[HALF:N_PAD, :]
        if "B" not in phases:
            NWIN_B = 0
        else:
            NWIN_B = NWIN
        import os as _os
        _bb = lambda k, d: int(_os.environ.get(k, d))
        with tc.tile_pool(name="b_glo", bufs=_bb("BGL", 3)) as glop, \
             tc.tile_pool(name="b_ghi", bufs=_bb("BGH", 3)) as ghip, \
             tc.tile_pool(name="b_ohg", bufs=_bb("BOG", 3)) as ohgp, \
             tc.tile_pool(name="b_oha", bufs=_bb("BOA", 3)) as ohap, \
             tc.tile_pool(name="b_sev", bufs=_bb("BSV", 4)) as sevp, \
             tc.tile_pool(name="b_e8", bufs=_bb("BE8", 2)) as e8p, \
             tc.tile_pool(name="b_md", bufs=_bb("BMD", 3)) as mdp, \
             tc.tile_pool(name="b_hw", bufs=_bb("BHW", 3)) as hwp, \
             tc.tile_pool(name="b_ev", bufs=_bb("BEV", 2), space="PSUM") as evp, \
             tc.tile_pool(name="b_sc", bufs=_bb("BSC", 2), space="PSUM") as scp, \
             tc.tile_pool(name="b_hu", bufs=_bb("BHU", 2), space="PSUM") as hup:

            icol_of = np.concatenate([[0], np.cumsum([8 * c for c in cpw])])
            _qctr = [0]

            def _next_q():
                q = _qctr[0] % GQN
                _qctr[0] += 1
                return q

            def win_gather(w):
                """Issue gathers + oh loads + one-hot builds for window w."""
                cb = cbase[w]
                icol = int(icol_of[w])
                tiles = {}
                ohg_w = ohgp.tile([P, CPWMX, P], f8, tag="ohg")
                if LVL >= 2:
                    nc.sync.dma_start(out=ohg_w[:, 0:cpw[w], :],
                                      in_=ohg_t[:, cb:cb + cpw[w], :])
                import os as _os2
                _SEQDMA = _os2.environ.get("DBG_SEQDMA") == "1"
                # >1024 idx per dma_gather call wedges the device
                MAXC = int(_os2.environ.get("MAXC", "8"))
                if lowC[w] and _SEQDMA:
                    G = glop.tile([P, LCMX, TBL_COLS], bf16, tag="Glo")
                    nc.sync.dma_start(
                        out=G[:, 0:lowC[w], :],
                        in_=fsv_t[0:lowC[w] * P, :].rearrange(
                            "(t p) f -> p t f", p=P))
                    tiles["lo"] = G
                    icol += 8 * lowC[w]
                if highC[w] and _SEQDMA:
                    G = ghip.tile([P, HCMX, TBL_COLS], bf16, tag="Ghi")
                    nc.sync.dma_start(
                        out=G[:, 0:highC[w], :],
                        in_=fsv_t[0:highC[w] * P, :].rearrange(
                            "(t p) f -> p t f", p=P))
                    tiles["hi"] = G
                    icol += 8 * highC[w]
                if lowC[w] and not _SEQDMA:
                    G = glop.tile([P, LCMX, TBL_COLS], bf16, tag="Glo")
                    o = 0
                    while o < (lowC[w] if LVL >= 1 else 0):
                        g = min(MAXC, lowC[w] - o)
                        ni = g * P
                        nc.gpsimd.dma_gather(
                            out_ap=G[:, o:o + g, :], in_ap=fsv_t[:, :],
                            idxs_ap=gidx[:, icol:icol + ni // 16],
                            num_idxs=ni, num_idxs_reg=ni, elem_size=TBL_COLS,
                            queue_num=_next_q())
                        icol += ni // 16
                        o += g
                    tiles["lo"] = G
                if highC[w] and not _SEQDMA:
                    G = ghip.tile([P, HCMX, TBL_COLS], bf16, tag="Ghi")
                    o = 0
                    while o < (highC[w] if LVL >= 1 else 0):
                        g = min(MAXC, highC[w] - o)
                        ni = g * P
                        nc.gpsimd.dma_gather(
                            out_ap=G[:, o:o + g, :], in_ap=fsv_hi,
                            idxs_ap=gidx[:, icol:icol + ni // 16],
                            num_idxs=ni, num_idxs_reg=ni, elem_size=TBL_COLS,
                            queue_num=_next_q())
                        icol += ni // 16
                        o += g
                    tiles["hi"] = G
                oha_w = ohap.tile([P, CPWMX, P], bf16, tag="oha")
                if LVL >= 2:
                    nc.vector.tensor_tensor(
                        out=oha_w[:, 0:cpw[w], :],
                        in0=iotab[:, None, :].to_broadcast([P, cpw[w], P]),
                        in1=dstf[:, cb:cb + cpw[w], None].to_broadcast(
                            [P, cpw[w], P]),
                        op=Alu.is_equal)
                return tiles, ohg_w, oha_w

            def chunk_of(w, c):
                """(section G-key, local idx) for chunk c of window w."""
                if c < lowC[w]:
                    return "lo", c
                return "hi", c - lowC[w]

            SG = int(_os.environ.get("BSG", "4"))  # chunks per silu group
            NPAR = int(_os.environ.get("BNP", "3"))  # windows in flight
            assert NPAR * CPWMX * NUM_HEADS * 4 <= 2048, "score tile > PSUM bank"

            def score_group(w, wi, tiles, ohg_w, score_ps, c0):
                gn = min(SG, cpw[w] - c0)
                ev2 = evp.tile([P, 2, SG, P], f32, tag="ev2")
                if LVL >= 3:
                    for i in range(gn):
                        sec, lc = chunk_of(w, c0 + i)
                        G = tiles[sec]
                        for b in range(2):
                            nc.tensor.matmul(
                                out=ev2[:, b, i, :],
                                lhsT=fd_slice[:, w, b * P:(b + 1) * P],
                                rhs=ohg_w[:, c0 + i, :], start=True, stop=False)
                            nc.tensor.matmul(
                                out=ev2[:, b, i, :],
                                lhsT=G[:, lc, b * P:(b + 1) * P],
                                rhs=identb[:], start=False, stop=True)
                sevT = sevp.tile([P, 2, SG, P], bf16, tag="sevT")
                if LVL >= 4:
                    nc.scalar.activation(out=sevT[:, :, 0:gn, :],
                                         in_=ev2[:, :, 0:gn, :], func=AF.Silu)
                for i in range(gn if LVL >= 5 else 0):
                    for b in range(2):
                        nc.tensor.matmul(
                            out=score_ps[:, wi, c0 + i, :],
                            lhsT=sevT[:, b, i, :], rhs=attnb[:, b, :],
                            start=(b == 0), stop=(b == 1))

            def win_finish(w, wi, tiles, ohg_w, oha_w, e8g):
                # md[:, c, 0:256] = fv * exp(score) broadcast; md[:, c, 256:264]
                # = exp(score) so ONE agg matmul also produces denominators.
                md = mdp.tile([P, CPWMX, AGG_COLS], bf16, tag="md")
                if LVL >= 5:
                    nc.vector.tensor_copy(out=md[:, 0:cpw[w], OUT_FEATS:],
                                          in_=e8g[:, wi, 0:cpw[w], :])
                for sec, g, off in (("lo", lowC[w], 0), ("hi", highC[w], lowC[w])):
                    if not g or LVL < 6:
                        continue
                    G = tiles[sec]
                    nc.vector.tensor_tensor(
                        out=md[:, off:off + g, 0:OUT_FEATS].rearrange(
                            "p c (f h) -> p c f h", h=NUM_HEADS),
                        in0=G[:, 0:g, OUT_FEATS:].rearrange(
                            "p c (f h) -> p c f h", h=NUM_HEADS),
                        in1=md[:, off:off + g, None, OUT_FEATS:].to_broadcast(
                            [P, g, FPH, NUM_HEADS]),
                        op=Alu.mult)
                # note: matmul accumulation groups must stay contiguous per
                # PSUM region on HW - interleaving two regions corrupts one.
                hw_s = hwp.tile([P, AGG_COLS], bf16, tag="hw_s")
                if LVL < 7:
                    return
                hupd = hup.tile([P, AGG_COLS], f32, tag="hupd")
                for c in range(cpw[w]):
                    nc.tensor.matmul(out=hupd[:, 0:AGG_COLS],
                                     lhsT=oha_w[:, c, :], rhs=md[:, c, :],
                                     start=(c == 0), stop=(c == cpw[w] - 1))
                nc.vector.tensor_copy(out=hw_s[:], in_=hupd[:])
                nc.sync.dma_start(out=hupd_t[w], in_=hw_s[:])

            LVL = int(_os.environ.get("DBG_LEVEL", "99"))
            w = 0
            while w < NWIN_B:
                grp = list(range(w, min(w + NPAR, NWIN_B)))
                nw = len(grp)
                score_ps = scp.tile([P, NPAR, CPWMX, NUM_HEADS], f32,
                                    tag="score")
                state = []
                for wi, ww in enumerate(grp):
                    tiles, ohg_w, oha_w = win_gather(ww)
                    state.append((ww, wi, tiles, ohg_w, oha_w))
                maxg = max((cpw[ww] + SG - 1) // SG for ww in grp)
                for gi in range(maxg):
                    for (ww, wi, tiles, ohg_w, oha_w) in state:
                        if gi * SG < cpw[ww]:
                            score_group(ww, wi, tiles, ohg_w, score_ps, gi * SG)
                # one exp per window-group: silu<->exp share no ACT table set,
                # so each transition costs a 1.3us table load - batch them.
                e8g = e8p.tile([P, NPAR, CPWMX, NUM_HEADS], bf16, tag="e8g")
                if LVL >= 5:
                    nc.scalar.activation(out=e8g[:, 0:nw], in_=score_ps[:, 0:nw],
                                         func=AF.Exp)
                for (ww, wi, tiles, ohg_w, oha_w) in state:
                    win_finish(ww, wi, tiles, ohg_w, oha_w, e8g)
                w += len(grp)

        # ---------------- Phase C: normalize + residual + silu ----------------
        with tc.tile_pool(name="c_sb", bufs=3) as sbp:
            for w0 in range(0, NWIN if "C" in phases else 0, ATILE):
                bt = min(ATILE, NWIN - w0)
                hu4 = sbp.tile([P, ATILE, AGG_COLS], bf16, tag="hu4")
                nc.sync.dma_start(out=hu4[:, 0:bt, :],
                                  in_=hupd_t[w0:w0 + bt].rearrange(
                                      "w p c -> p w c"))
                F4 = sbp.tile([P, ATILE, IN_FEATS], bf16, tag="F4")
                nc.sync.dma_start(
                    out=F4[:, 0:bt, :],
                    in_=featmy_t[w0 * P:(w0 + bt) * P, :].rearrange(
                        "(t p) f -> p t f", p=P))
                h4 = sbp.tile([P, ATILE, IN_FEATS], bf16, tag="h")
                for i in range(bt):
                    w = w0 + i
                    nc.vector.tensor_scalar(
                        out=h4[:, i, :], in0=F4[:, i, :],
                        scalar1=stats_my[:, w, 0:1],
                        scalar2=stats_my[:, w, 1:2],
                        op0=Alu.subtract, op1=Alu.mult)
                den = sbp.tile([P, ATILE, NUM_HEADS], f32, tag="den")
                nc.vector.tensor_scalar_add(out=den[:, 0:bt],
                                            in0=hu4[:, 0:bt, OUT_FEATS:],
                                            scalar1=1e-30)
                denr = sbp.tile([P, ATILE, NUM_HEADS], f32, tag="denr")
                nc.vector.reciprocal(out=denr[:, 0:bt], in_=den[:, 0:bt])
                o = sbp.tile([P, ATILE, FPH, NUM_HEADS], f32, tag="o")
                nc.vector.tensor_tensor(
                    out=o[:, 0:bt],
                    in0=hu4[:, 0:bt, 0:OUT_FEATS].rearrange(
                        "p w (f h) -> p w f h", h=NUM_HEADS),
                    in1=denr[:, 0:bt, None, :].to_broadcast(
                        [P, bt, FPH, NUM_HEADS]),
                    op=Alu.mult)
                on = sbp.tile([P, ATILE, OUT_FEATS], f32, tag="on")
                nc.vector.tensor_tensor(
                    out=on[:, 0:bt].rearrange("p w (h f) -> p w h f",
                                              h=NUM_HEADS),
                    in0=o[:, 0:bt].rearrange("p w f h -> p w h f"),
                    in1=h4[:, 0:bt].rearrange("p w (h f) -> p w h f",
                                              h=NUM_HEADS),
                    op=Alu.add)
                oo = sbp.tile([P, ATILE, OUT_FEATS], bf16, tag="oo")
                nc.scalar.activation(out=oo[:, 0:bt], in_=on[:, 0:bt],
                                     func=AF.Silu)
                nc.sync.dma_start(
                    out=out_t[w0 * P:(w0 + bt) * P, :].rearrange(
                        "(w p) f -> p w f", p=P),
                    in_=oo[:, 0:bt])

    nc.compile()
    return nc, (lowC, highC)


def _derive_schedule(src, dst):
    """Per-window chunk counts (low/high table half), maxed over cores."""
    lowC = np.zeros(NWIN, np.int64)
    highC = np.zeros(NWIN, np.int64)
    for core in range(N_CORES):
        lo, hi = core * SLICE, (core + 1) * SLICE
        m = (dst >= lo) & (dst < hi)
        w_of = (dst[m] - lo) // P
        is_lo = src[m] < HALF
        cl = np.bincount(w_of[is_lo], minlength=NWIN)
        ch = np.bincount(w_of[~is_lo], minlength=NWIN)
        lowC = np.maximum(lowC, (cl + P - 1) // P)
        highC = np.maximum(highC, (ch + P - 1) // P)
    return tuple(int(x) for x in lowC), tuple(int(x) for x in highC)


def _prepare_core_inputs(core, src, dst, lowC, highC):
    import ml_dtypes
    cpw = [l + h for l, h in zip(lowC, highC)]
    C_TOT = sum(cpw)
    cbase = np.concatenate([[0], np.cumsum(cpw)]).astype(int)

    lo, hi = core * SLICE, (core + 1) * SLICE
    m = (dst >= lo) & (dst < hi)
    dsl = dst[m] - lo
    ssl = src[m]
    w_of = dsl // P
    is_lo = ssl < HALF

    slot_src = np.zeros((C_TOT, P), np.int64)
    slot_doff = np.full((C_TOT, P), -1.0, np.float32)

    # sort each (window, table-half) section by src so the gather reads the
    # table in ascending row order (HBM locality)
    order = np.lexsort((ssl, ~is_lo, w_of))
    ssl_o, dsl_o, w_o, lo_o = ssl[order], dsl[order], w_of[order], is_lo[order]
    for w in range(NWIN):
        wm = w_o == w
        for half, cb, g in ((True, cbase[w], lowC[w]),
                            (False, cbase[w] + lowC[w], highC[w])):
            sel = wm & (lo_o == half)
            s_w = ssl_o[sel]
            d_w = dsl_o[sel] - w * P
            n = len(s_w)
            assert n <= g * P, (core, w, half, n, g * P)
            flat_s = slot_src[cb:cb + g].reshape(-1)
            flat_d = slot_doff[cb:cb + g].reshape(-1)
            flat_s[:n] = s_w
            flat_d[:n] = d_w

    dstf = slot_doff.T.copy().astype(ml_dtypes.bfloat16)  # [P, C_TOT]

    ohg = np.zeros((P, C_TOT, P), ml_dtypes.float8_e4m3)
    cc, ee = np.nonzero(slot_doff >= 0)
    ohg[slot_doff[cc, ee].astype(np.int64), cc, ee] = 1

    idx_cols = []
    for w in range(NWIN):
        for half, cb, g in ((True, cbase[w], lowC[w]),
                            (False, cbase[w] + lowC[w], highC[w])):
            if not g:
                continue
            s = slot_src[cb:cb + g].reshape(-1).copy()
            if not half:
                s = np.maximum(s - HALF, 0)
            idx = s.astype(np.int16)
            idx_cols.append(np.tile(idx.reshape(-1, 16).T, (8, 1)))
    gidx = np.concatenate(idx_cols, axis=1).astype(np.int16)
    return dstf, gidx, ohg


def _shared_inputs(inputs):
    import ml_dtypes
    feat = np.asarray(inputs["feat"], np.float32)
    Wsrc = np.asarray(inputs["Wsrc"], np.float32)
    Wdst = np.asarray(inputs["Wdst"], np.float32)
    Wval = np.asarray(inputs["Wval"], np.float32)
    attn = np.asarray(inputs["attn"], np.float32).reshape(NUM_HEADS, FPH)

    featb = np.zeros((N_PAD, IN_FEATS), ml_dtypes.bfloat16)
    featb[:N_NODES] = feat.astype(ml_dtypes.bfloat16)
    # pre-transposed/tiled: [tile, f-partition, block, node]
    featTb = np.ascontiguousarray(
        featb.reshape(NTILES, P, 2, P).transpose(0, 3, 2, 1))

    # weights transposed + output-column permuted to (f,h)-major.
    # LayerNorm mean-fold: h@W^T = rstd*(feat@W'^T) with W' = W - rowmean(W)
    # (the -mu*ones part of LN folds into centered weight rows; rstd is
    # applied per-node after the GEMM).
    WsrcP = Wsrc[_OLD_OF_NEW, :]   # [256 newcol, 256 in]
    WvalP = Wval[_OLD_OF_NEW, :]
    WdstP = Wdst[_OLD_OF_NEW, :]
    WsrcP = WsrcP - WsrcP.mean(axis=1, keepdims=True)
    WvalP = WvalP - WvalP.mean(axis=1, keepdims=True)
    WdstP = WdstP - WdstP.mean(axis=1, keepdims=True)
    wfsv = np.zeros((P, 2, TBL_COLS), np.float32)
    for b in range(2):
        wfsv[:, b, 0:OUT_FEATS] = WsrcP[:, b * P:(b + 1) * P].T
        wfsv[:, b, OUT_FEATS:] = WvalP[:, b * P:(b + 1) * P].T
    wfd = np.zeros((P, 2, OUT_FEATS), np.float32)
    for b in range(2):
        wfd[:, b, :] = WdstP[:, b * P:(b + 1) * P].T

    attnb = np.zeros((P, 2, NUM_HEADS), np.float32)
    for b in range(2):
        j = b * P + np.arange(P)
        attnb[np.arange(P), b, j % NUM_HEADS] = attn[j % NUM_HEADS,
                                                     j // NUM_HEADS]

    identb = np.eye(P, dtype=ml_dtypes.bfloat16)
    iotab = np.tile(np.arange(P, dtype=ml_dtypes.bfloat16).reshape(1, P),
                    (P, 1))
    bf = ml_dtypes.bfloat16
    return (featTb, feat, wfsv.astype(bf), wfd.astype(bf), attnb.astype(bf),
            identb, iotab)


def make_in_maps(inputs, lowC, highC):
    import ml_dtypes
    featTb, feat, wfsv, wfd, attnb, identb, iotab = _shared_inputs(inputs)
    src = np.asarray(inputs["src"], np.int64)
    dst = np.asarray(inputs["dst"], np.int64)
    in_maps = []
    for core in range(N_CORES):
        dstf, gidx, ohg = _prepare_core_inputs(core, src, dst, lowC, highC)
        featmy = np.zeros((SLICE_PAD, IN_FEATS), ml_dtypes.bfloat16)
        n = min(SLICE_PAD, N_NODES - core * SLICE)
        featmy[:n] = feat[core * SLICE:core * SLICE + n].astype(
            ml_dtypes.bfloat16)
        featmyT = np.ascontiguousarray(
            featmy.reshape(NWIN, P, 2, P).transpose(0, 3, 2, 1))
        in_maps.append(dict(
            featTb=featTb, featmyT=featmyT, featmy=featmy, wfsv=wfsv, wfd=wfd,
            attnb=attnb, identb=identb, iotab=iotab, dstf=dstf, gidx=gidx,
            ohg=ohg,
        ))
    return in_maps


def kernel(**inputs):
    import concourse.bass_utils as bass_utils

    for b in ("bsrc", "bdst", "bval"):
        assert not np.any(np.asarray(inputs[b])), \
            "nonzero biases unsupported by this kernel"
    src = np.asarray(inputs["src"], np.int64)
    dst = np.asarray(inputs["dst"], np.int64)

    lowC, highC = _derive_schedule(src, dst)
    key = (lowC, highC)
    if key not in _CACHE:
        _CACHE[key] = _build_nc(lowC, highC)
    nc, _ = _CACHE[key]

    in_maps = make_in_maps(inputs, lowC, highC)
    res = bass_utils.run_bass_kernel_spmd(nc, in_maps, core_ids=list(range(N_CORES)))
    out = np.concatenate(
        [res.results[c]["outmy"][:SLICE] for c in range(N_CORES)], axis=0)
    return np.ascontiguousarray(out.astype(np.float32))



# revision 36
# speedup vs baseline: 1.6082x; 1.0117x over previous
"""GATv3Conv Trainium2 kernel (8 NeuronCores, SPMD).

Strategy (v4):
  - Shard EDGES by destination-node slice (core k owns dst in [k*6250,(k+1)*6250)).
    Segment softmax + aggregation are fully core-local (no collectives).
  - LayerNorm MEAN is folded into centered weight rows on host
    (h@W^T = rstd*(feat@W'^T), W' = W - rowmean(W)); rstd is applied to the
    GEMM output per node (DVE fs-half, ACT fv-half).
  - feat arrives HOST-pre-transposed/tiled ([tile, f-part, blk, node]) so
    phase A needs no natural-layout load and no on-device transpose; LN stats
    come from ACT Square + PE ones-matmuls (sum/sumsq in PSUM).
  - Each core redundantly computes the src/val GEMM table [N,512] = [fs|fv]
    bf16 in its HBM ((f,h)-major cols); dst GEMM only for its slice (SBUF).
  - Edge phase, per 128-dst-node window, edges in 128-edge chunks:
      * fsv rows gathered via gpsimd.dma_gather, round-robined over 4 SWDGE
        queues (single queue serializes the drain: 715us -> ~340us);
        4 G-pool bufs give the gather stream lookahead.
      * evT[f,e] = (one-hot dst gather of fd via matmul; one-hots in fp8) +
        (fs^T via identity matmul), accumulated in PSUM; silu on ACT.
      * score[e,h] via PE: lhsT=sevT half, rhs=block-diag attn [128,8].
      * ONE Exp per NPAR(3)-window group (shared score PSUM tile): silu and
        exp share no ACT table set, so each transition costs a 1.3us table
        load - batch them.
      * md[:,c,0:256] = fv*e8 (DVE bf16 2x broadcast); md[:,c,256:264] = e8 so
        a single one-hot agg matmul chain also produces denominators ([P,264]).
  - Softmax division deferred to batched phase C: out = silu(num/den + h),
    written bf16. exp() uses raw scores (no segment max, scores O(+-10)).
"""

import numpy as np

N_NODES = 50000
IN_FEATS = 256
OUT_FEATS = 256
NUM_HEADS = 8
FPH = OUT_FEATS // NUM_HEADS   # 32
LN_EPS = 1e-5
N_CORES = 8
SLICE = N_NODES // N_CORES     # 6250
P = 128
NWIN = (SLICE + P - 1) // P    # 49
SLICE_PAD = NWIN * P           # 6272
N_PAD = ((N_NODES + P - 1) // P) * P   # 50048
NTILES = N_PAD // P            # 391
HALF = 32768
TBL_COLS = 2 * OUT_FEATS       # 512
AGG_COLS = OUT_FEATS + NUM_HEADS  # 264
import os as _osm
ATILE = int(_osm.environ.get("ATILE", "4"))  # node tiles per phase-A DMA batch

# new feature order is (f, h)-major: new col j=f*8+h <- old col h*32+f
_OLD_OF_NEW = (np.arange(OUT_FEATS) % NUM_HEADS) * FPH + \
    np.arange(OUT_FEATS) // NUM_HEADS

_CACHE = {}


def _build_nc(lowC, highC, reps=1, phases="ABC"):
    import concourse.bacc as bacc
    import concourse.tile as tile
    from concourse import mybir
    from contextlib import ExitStack

    f32 = mybir.dt.float32
    bf16 = mybir.dt.bfloat16
    f8 = mybir.dt.float8e4
    i16 = mybir.dt.int16
    AF = mybir.ActivationFunctionType
    Alu = mybir.AluOpType

    lowC = list(lowC)
    highC = list(highC)
    cpw = [l + h for l, h in zip(lowC, highC)]
    cbase = np.concatenate([[0], np.cumsum(cpw)]).astype(int)
    C_TOT = int(cbase[-1])
    CPWMX = max(cpw)
    LCMX = max(lowC)
    HCMX = max(max(highC), 1)
    icols = 8 * C_TOT  # int16 idx cols (128 idx -> 8 cols of 16)

    import os as _os0
    GQN = int(_os0.environ.get("GQN", "4"))
    nc = bacc.Bacc(None, target_bir_lowering=False, num_swdge_queues=GQN)

    featTb_t = nc.dram_tensor("featTb", [NTILES, P, 2, P], bf16,
                              kind="ExternalInput")
    featmyT_t = nc.dram_tensor("featmyT", [NWIN, P, 2, P], bf16,
                               kind="ExternalInput")
    featmy_t = nc.dram_tensor("featmy", [SLICE_PAD, IN_FEATS], bf16,
                              kind="ExternalInput")
    wfsv_t = nc.dram_tensor("wfsv", [P, 2, TBL_COLS], bf16, kind="ExternalInput")
    wfd_t = nc.dram_tensor("wfd", [P, 2, OUT_FEATS], bf16, kind="ExternalInput")
    attnb_t = nc.dram_tensor("attnb", [P, 2, NUM_HEADS], bf16, kind="ExternalInput")
    identb_t = nc.dram_tensor("identb", [P, P], bf16, kind="ExternalInput")
    iotab_t = nc.dram_tensor("iotab", [P, P], bf16, kind="ExternalInput")
    dstf_t = nc.dram_tensor("dstf", [P, C_TOT], bf16, kind="ExternalInput")
    gidx_t = nc.dram_tensor("gidx", [P, icols], i16, kind="ExternalInput")
    ohg_t = nc.dram_tensor("ohg", [P, C_TOT, P], f8, kind="ExternalInput")
    out_t = nc.dram_tensor("outmy", [SLICE_PAD, OUT_FEATS], bf16,
                           kind="ExternalOutput")

    fsv_t = nc.dram_tensor("fsvtbl", [N_PAD, TBL_COLS], bf16, kind="Internal")
    hupd_t = nc.dram_tensor("hupdtbl", [NWIN, P, AGG_COLS], bf16, kind="Internal")

    with tile.TileContext(nc) as tc, ExitStack() as ctx:
        if reps > 1:
            ctx.enter_context(tc.For_i(0, reps, 1))
        const = ctx.enter_context(tc.tile_pool(name="const", bufs=1))
        persist = ctx.enter_context(tc.tile_pool(name="persist", bufs=1))

        wfsv = const.tile([P, 2, TBL_COLS], bf16)
        nc.sync.dma_start(out=wfsv, in_=wfsv_t[:, :, :])
        wfd = const.tile([P, 2, OUT_FEATS], bf16)
        nc.sync.dma_start(out=wfd, in_=wfd_t[:, :, :])
        attnb = const.tile([P, 2, NUM_HEADS], bf16)
        nc.sync.dma_start(out=attnb, in_=attnb_t[:, :, :])
        identb = const.tile([P, P], bf16)
        nc.sync.dma_start(out=identb, in_=identb_t[:, :])
        iotab = const.tile([P, P], bf16)
        nc.sync.dma_start(out=iotab, in_=iotab_t[:, :])
        dstf = const.tile([P, C_TOT], bf16)
        nc.sync.dma_start(out=dstf, in_=dstf_t[:, :])
        gidx = const.tile([P, icols], i16)
        nc.sync.dma_start(out=gidx, in_=gidx_t[:, :])
        eps_c = const.tile([P, 1], f32)
        nc.vector.memset(eps_c[:], LN_EPS)
        ones_c = const.tile([P, 1], bf16)
        nc.vector.memset(ones_c[:], 1.0)

        fd_slice = persist.tile([P, NWIN, OUT_FEATS], bf16)
        stats_my = persist.tile([P, NWIN, 2], f32)   # (mean, rstd)

        # ---------------- Phase A: LN + GEMM tables ----------------
        # feat arrives pre-transposed/tiled from host ([tile, f-part, blk, node]).
        # LN mean is folded into centered weights; stats (sum/sumsq) come from
        # ones-matmuls on hT and ACT Square, so no natural-layout load at all.
        def node_sweep(srcT_dram, ntiles, emit, wtile, wcols):
            import os as _osA
            with tc.tile_pool(name="a_sb", bufs=int(_osA.environ.get("ASB", "4"))) as sbp, \
                 tc.tile_pool(name="a_ps", bufs=int(_osA.environ.get("APS", "4")), space="PSUM") as psp, \
                 tc.tile_pool(name="a_st", bufs=int(_osA.environ.get("AST", "3")), space="PSUM") as stp:
                for t0 in range(0, ntiles, ATILE):
                    bt = min(ATILE, ntiles - t0)
                    hT4 = sbp.tile([P, ATILE, 2, P], bf16, tag="hT4")
                    nc.sync.dma_start(
                        out=hT4[:, 0:bt, :, :],
                        in_=srcT_dram[t0:t0 + bt].rearrange("t p b e -> p t b e"))
                    sq4 = sbp.tile([P, ATILE, 2, P], bf16, tag="sq4")
                    nc.scalar.activation(out=sq4[:, 0:bt], in_=hT4[:, 0:bt],
                                         func=AF.Square)
                    sps = stp.tile([P, ATILE, 2], f32, tag="sps")
                    for i in range(bt):
                        for b in range(2):
                            nc.tensor.matmul(out=sps[:, i, 0:1],
                                             lhsT=hT4[:, i, b, :], rhs=ones_c[:],
                                             start=(b == 0), stop=(b == 1))
                        for b in range(2):
                            nc.tensor.matmul(out=sps[:, i, 1:2],
                                             lhsT=sq4[:, i, b, :], rhs=ones_c[:],
                                             start=(b == 0), stop=(b == 1))
                    # var = sumsq/256 - (sum/256)^2 ; rstd = 1/sqrt(var+eps)
                    mean4 = sbp.tile([P, ATILE], f32, tag="mean4")
                    nc.vector.tensor_scalar(out=mean4[:, 0:bt],
                                            in0=sps[:, 0:bt, 0],
                                            scalar1=1.0 / IN_FEATS, scalar2=None,
                                            op0=Alu.mult)
                    mu2 = sbp.tile([P, ATILE], f32, tag="mu2")
                    nc.scalar.activation(out=mu2[:, 0:bt], in_=sps[:, 0:bt, 0],
                                         func=AF.Square, scale=1.0 / IN_FEATS)
                    var4 = sbp.tile([P, ATILE], f32, tag="var4")
                    nc.vector.tensor_scalar(out=var4[:, 0:bt],
                                            in0=sps[:, 0:bt, 1],
                                            scalar1=1.0 / IN_FEATS,
                                            scalar2=None, op0=Alu.mult)
                    vc4 = sbp.tile([P, ATILE], f32, tag="vc4")
                    nc.vector.tensor_tensor(out=vc4[:, 0:bt], in0=var4[:, 0:bt],
                                            in1=mu2[:, 0:bt], op=Alu.subtract)
                    sd4 = sbp.tile([P, ATILE], f32, tag="sd4")
                    nc.scalar.activation(out=sd4[:, 0:bt], in_=vc4[:, 0:bt],
                                         func=AF.Sqrt, bias=eps_c[:])
                    rstd4 = sbp.tile([P, ATILE], f32, tag="rstd4")
                    nc.vector.reciprocal(out=rstd4[:, 0:bt], in_=sd4[:, 0:bt])
                    batch = []
                    for i in range(bt):
                        g = psp.tile([P, wcols], f32, tag="gemm")
                        nc.tensor.matmul(out=g[:], lhsT=hT4[:, i, 0, :],
                                         rhs=wtile[:, 0, :], start=True, stop=False)
                        nc.tensor.matmul(out=g[:], lhsT=hT4[:, i, 1, :],
                                         rhs=wtile[:, 1, :], start=False, stop=True)
                        emit(t0 + i, i, g, mean4, rstd4, sbp, batch)
                    if batch:
                        st4, bt0 = batch[0]
                        nc.sync.dma_start(
                            out=fsv_t[bt0 * P:(bt0 + bt) * P, :].rearrange(
                                "(t p) f -> p t f", p=P),
                            in_=st4[:, 0:bt, :])

        def emit_fsv(t, i, g, mean4, rstd4, sbp, batch):
            if i == 0:
                st4 = sbp.tile([P, ATILE, TBL_COLS], bf16, tag="fsv4")
                batch.append((st4, t))
            st4, _ = batch[0]
            nc.vector.tensor_scalar(
                out=st4[:, i, 0:OUT_FEATS], in0=g[:, 0:OUT_FEATS],
                scalar1=rstd4[:, i:i + 1], scalar2=None, op0=Alu.mult)
            nc.scalar.activation(out=st4[:, i, OUT_FEATS:], in_=g[:, OUT_FEATS:],
                                 func=AF.Copy, scale=rstd4[:, i:i + 1])

        def emit_fd(t, i, g, mean4, rstd4, sbp, batch):
            nc.vector.tensor_scalar(
                out=fd_slice[:, t, 0:P], in0=g[:, 0:P],
                scalar1=rstd4[:, i:i + 1], scalar2=None, op0=Alu.mult)
            nc.scalar.activation(out=fd_slice[:, t, P:OUT_FEATS], in_=g[:, P:],
                                 func=AF.Copy, scale=rstd4[:, i:i + 1])
            nc.vector.tensor_copy(out=stats_my[:, t, 0:1], in_=mean4[:, i:i + 1])
            nc.vector.tensor_copy(out=stats_my[:, t, 1:2],
                                  in_=rstd4[:, i:i + 1])

        node_sweep(featmyT_t, NWIN, emit_fd, wfd, OUT_FEATS)
        if "A" in phases:
            node_sweep(featTb_t, NTILES, emit_fsv, wfsv, TBL_COLS)

        # ---------------- Phase B: edge phase ----------------
        fsv_hi = fsv_t[HALF:N_PAD, :]
        if "B" not in phases:
            NWIN_B = 0
        else:
            NWIN_B = NWIN
        import os as _os
        _bb = lambda k, d: int(_os.environ.get(k, d))
        with tc.tile_pool(name="b_glo", bufs=_bb("BGL", 4)) as glop, \
             tc.tile_pool(name="b_ghi", bufs=_bb("BGH", 4)) as ghip, \
             tc.tile_pool(name="b_ohg", bufs=_bb("BOG", 3)) as ohgp, \
             tc.tile_pool(name="b_oha", bufs=_bb("BOA", 3)) as ohap, \
             tc.tile_pool(name="b_sev", bufs=_bb("BSV", 4)) as sevp, \
             tc.tile_pool(name="b_e8", bufs=_bb("BE8", 2)) as e8p, \
             tc.tile_pool(name="b_md", bufs=_bb("BMD", 3)) as mdp, \
             tc.tile_pool(name="b_hw", bufs=_bb("BHW", 3)) as hwp, \
             tc.tile_pool(name="b_ev", bufs=_bb("BEV", 2), space="PSUM") as evp, \
             tc.tile_pool(name="b_sc", bufs=_bb("BSC", 2), space="PSUM") as scp, \
             tc.tile_pool(name="b_hu", bufs=_bb("BHU", 2), space="PSUM") as hup:

            icol_of = np.concatenate([[0], np.cumsum([8 * c for c in cpw])])
            _qctr = [0]

            def _next_q():
                q = _qctr[0] % GQN
                _qctr[0] += 1
                return q

            def win_gather(w):
                """Issue gathers + oh loads + one-hot builds for window w."""
                cb = cbase[w]
                icol = int(icol_of[w])
                tiles = {}
                ohg_w = ohgp.tile([P, CPWMX, P], f8, tag="ohg")
                if LVL >= 2:
                    nc.sync.dma_start(out=ohg_w[:, 0:cpw[w], :],
                                      in_=ohg_t[:, cb:cb + cpw[w], :])
                import os as _os2
                _SEQDMA = _os2.environ.get("DBG_SEQDMA") == "1"
                # >1024 idx per dma_gather call wedges the device
                MAXC = int(_os2.environ.get("MAXC", "8"))
                if lowC[w] and _SEQDMA:
                    G = glop.tile([P, LCMX, TBL_COLS], bf16, tag="Glo")
                    nc.sync.dma_start(
                        out=G[:, 0:lowC[w], :],
                        in_=fsv_t[0:lowC[w] * P, :].rearrange(
                            "(t p) f -> p t f", p=P))
                    tiles["lo"] = G
                    icol += 8 * lowC[w]
                if highC[w] and _SEQDMA:
                    G = ghip.tile([P, HCMX, TBL_COLS], bf16, tag="Ghi")
                    nc.sync.dma_start(
                        out=G[:, 0:highC[w], :],
                        in_=fsv_t[0:highC[w] * P, :].rearrange(
                            "(t p) f -> p t f", p=P))
                    tiles["hi"] = G
                    icol += 8 * highC[w]
                if lowC[w] and not _SEQDMA:
                    G = glop.tile([P, LCMX, TBL_COLS], bf16, tag="Glo")
                    o = 0
                    while o < (lowC[w] if LVL >= 1 else 0):
                        g = min(MAXC, lowC[w] - o)
                        ni = g * P
                        nc.gpsimd.dma_gather(
                            out_ap=G[:, o:o + g, :], in_ap=fsv_t[:, :],
                            idxs_ap=gidx[:, icol:icol + ni // 16],
                            num_idxs=ni, num_idxs_reg=ni, elem_size=TBL_COLS,
                            queue_num=_next_q())
                        icol += ni // 16
                        o += g
                    tiles["lo"] = G
                if highC[w] and not _SEQDMA:
                    G = ghip.tile([P, HCMX, TBL_COLS], bf16, tag="Ghi")
                    o = 0
                    while o < (highC[w] if LVL >= 1 else 0):
                        g = min(MAXC, highC[w] - o)
                        ni = g * P
                        nc.gpsimd.dma_gather(
                            out_ap=G[:, o:o + g, :], in_ap=fsv_hi,
                            idxs_ap=gidx[:, icol:icol + ni // 16],
                            num_idxs=ni, num_idxs_reg=ni, elem_size=TBL_COLS,
                            queue_num=_next_q())
                        icol += ni // 16
                        o += g
                    tiles["hi"] = G
                oha_w = ohap.tile([P, CPWMX, P], bf16, tag="oha")
                if LVL >= 2:
                    nc.vector.tensor_tensor(
                        out=oha_w[:, 0:cpw[w], :],
                        in0=iotab[:, None, :].to_broadcast([P, cpw[w], P]),
                        in1=dstf[:, cb:cb + cpw[w], None].to_broadcast(
                            [P, cpw[w], P]),
                        op=Alu.is_equal)
                return tiles, ohg_w, oha_w

            def chunk_of(w, c):
                """(section G-key, local idx) for chunk c of window w."""
                if c < lowC[w]:
                    return "lo", c
                return "hi", c - lowC[w]

            SG = int(_os.environ.get("BSG", "4"))  # chunks per silu group
            NPAR = int(_os.environ.get("BNP", "3"))  # windows in flight
            assert NPAR * CPWMX * NUM_HEADS * 4 <= 2048, "score tile > PSUM bank"

            def score_group(w, wi, tiles, ohg_w, score_ps, c0):
                gn = min(SG, cpw[w] - c0)
                ev2 = evp.tile([P, 2, SG, P], f32, tag="ev2")
                if LVL >= 3:
                    for i in range(gn):
                        sec, lc = chunk_of(w, c0 + i)
                        G = tiles[sec]
                        for b in range(2):
                            nc.tensor.matmul(
                                out=ev2[:, b, i, :],
                                lhsT=fd_slice[:, w, b * P:(b + 1) * P],
                                rhs=ohg_w[:, c0 + i, :], start=True, stop=False)
                            nc.tensor.matmul(
                                out=ev2[:, b, i, :],
                                lhsT=G[:, lc, b * P:(b + 1) * P],
                                rhs=identb[:], start=False, stop=True)
                sevT = sevp.tile([P, 2, SG, P], bf16, tag="sevT")
                if LVL >= 4:
                    nc.scalar.activation(out=sevT[:, :, 0:gn, :],
                                         in_=ev2[:, :, 0:gn, :], func=AF.Silu)
                for i in range(gn if LVL >= 5 else 0):
                    for b in range(2):
                        nc.tensor.matmul(
                            out=score_ps[:, wi, c0 + i, :],
                            lhsT=sevT[:, b, i, :], rhs=attnb[:, b, :],
                            start=(b == 0), stop=(b == 1))

            def win_finish(w, wi, tiles, ohg_w, oha_w, e8g):
                # md[:, c, 0:256] = fv * exp(score) broadcast; md[:, c, 256:264]
                # = exp(score) so ONE agg matmul also produces denominators.
                md = mdp.tile([P, CPWMX, AGG_COLS], bf16, tag="md")
                if LVL >= 5:
                    nc.vector.tensor_copy(out=md[:, 0:cpw[w], OUT_FEATS:],
                                          in_=e8g[:, wi, 0:cpw[w], :])
                for sec, g, off in (("lo", lowC[w], 0), ("hi", highC[w], lowC[w])):
                    if not g or LVL < 6:
                        continue
                    G = tiles[sec]
                    nc.vector.tensor_tensor(
                        out=md[:, off:off + g, 0:OUT_FEATS].rearrange(
                            "p c (f h) -> p c f h", h=NUM_HEADS),
                        in0=G[:, 0:g, OUT_FEATS:].rearrange(
                            "p c (f h) -> p c f h", h=NUM_HEADS),
                        in1=md[:, off:off + g, None, OUT_FEATS:].to_broadcast(
                            [P, g, FPH, NUM_HEADS]),
                        op=Alu.mult)
                # note: matmul accumulation groups must stay contiguous per
                # PSUM region on HW - interleaving two regions corrupts one.
                hw_s = hwp.tile([P, AGG_COLS], bf16, tag="hw_s")
                if LVL < 7:
                    return
                hupd = hup.tile([P, AGG_COLS], f32, tag="hupd")
                for c in range(cpw[w]):
                    nc.tensor.matmul(out=hupd[:, 0:AGG_COLS],
                                     lhsT=oha_w[:, c, :], rhs=md[:, c, :],
                                     start=(c == 0), stop=(c == cpw[w] - 1))
                nc.vector.tensor_copy(out=hw_s[:], in_=hupd[:])
                nc.sync.dma_start(out=hupd_t[w], in_=hw_s[:])

            LVL = int(_os.environ.get("DBG_LEVEL", "99"))
            w = 0
            while w < NWIN_B:
                grp = list(range(w, min(w + NPAR, NWIN_B)))
                nw = len(grp)
                score_ps = scp.tile([P, NPAR, CPWMX, NUM_HEADS], f32,
                                    tag="score")
                state = []
                for wi, ww in enumerate(grp):
                    tiles, ohg_w, oha_w = win_gather(ww)
                    state.append((ww, wi, tiles, ohg_w, oha_w))
                maxg = max((cpw[ww] + SG - 1) // SG for ww in grp)
                for gi in range(maxg):
                    for (ww, wi, tiles, ohg_w, oha_w) in state:
                        if gi * SG < cpw[ww]:
                            score_group(ww, wi, tiles, ohg_w, score_ps, gi * SG)
                # one exp per window-group: silu<->exp share no ACT table set,
                # so each transition costs a 1.3us table load - batch them.
                e8g = e8p.tile([P, NPAR, CPWMX, NUM_HEADS], bf16, tag="e8g")
                if LVL >= 5:
                    nc.scalar.activation(out=e8g[:, 0:nw], in_=score_ps[:, 0:nw],
                                         func=AF.Exp)
                for (ww, wi, tiles, ohg_w, oha_w) in state:
                    win_finish(ww, wi, tiles, ohg_w, oha_w, e8g)
                w += len(grp)

        # ---------------- Phase C: normalize + residual + silu ----------------
        with tc.tile_pool(name="c_sb", bufs=3) as sbp:
            for w0 in range(0, NWIN if "C" in phases else 0, ATILE):
                bt = min(ATILE, NWIN - w0)
                hu4 = sbp.tile([P, ATILE, AGG_COLS], bf16, tag="hu4")
                nc.sync.dma_start(out=hu4[:, 0:bt, :],
                                  in_=hupd_t[w0:w0 + bt].rearrange(
                                      "w p c -> p w c"))
                F4 = sbp.tile([P, ATILE, IN_FEATS], bf16, tag="F4")
                nc.sync.dma_start(
                    out=F4[:, 0:bt, :],
                    in_=featmy_t[w0 * P:(w0 + bt) * P, :].rearrange(
                        "(t p) f -> p t f", p=P))
                h4 = sbp.tile([P, ATILE, IN_FEATS], bf16, tag="h")
                for i in range(bt):
                    w = w0 + i
                    nc.vector.tensor_scalar(
                        out=h4[:, i, :], in0=F4[:, i, :],
                        scalar1=stats_my[:, w, 0:1],
                        scalar2=stats_my[:, w, 1:2],
                        op0=Alu.subtract, op1=Alu.mult)
                den = sbp.tile([P, ATILE, NUM_HEADS], f32, tag="den")
                nc.vector.tensor_scalar_add(out=den[:, 0:bt],
                                            in0=hu4[:, 0:bt, OUT_FEATS:],
                                            scalar1=1e-30)
                denr = sbp.tile([P, ATILE, NUM_HEADS], f32, tag="denr")
                nc.vector.reciprocal(out=denr[:, 0:bt], in_=den[:, 0:bt])
                o = sbp.tile([P, ATILE, FPH, NUM_HEADS], f32, tag="o")
                nc.vector.tensor_tensor(
                    out=o[:, 0:bt],
                    in0=hu4[:, 0:bt, 0:OUT_FEATS].rearrange(
                        "p w (f h) -> p w f h", h=NUM_HEADS),
                    in1=denr[:, 0:bt, None, :].to_broadcast(
                        [P, bt, FPH, NUM_HEADS]),
                    op=Alu.mult)
                on = sbp.tile([P, ATILE, OUT_FEATS], f32, tag="on")
                nc.vector.tensor_tensor(
                    out=on[:, 0:bt].rearrange("p w (h f) -> p w h f",
                                              h=NUM_HEADS),
                    in0=o[:, 0:bt].rearrange("p w f h -> p w h f"),
                    in1=h4[:, 0:bt].rearrange("p w (h f) -> p w h f",
                                              h=NUM_HEADS),
                    op=Alu.add)
                oo = sbp.tile([P, ATILE, OUT_FEATS], bf16, tag="oo")
                nc.scalar.activation(out=oo[:, 0:bt], in_=on[:, 0:bt],
                                     func=AF.Silu)
                nc.sync.dma_start(
                    out=out_t[w0 * P:(w0 + bt) * P, :].rearrange(
                        "(w p) f -> p w f", p=P),
                    in_=oo[:, 0:bt])

    nc.compile()
    return nc, (lowC, highC)


def _derive_schedule(src, dst):
    """Per-window chunk counts (low/high table half), maxed over cores."""
    lowC = np.zeros(NWIN, np.int64)
    highC = np.zeros(NWIN, np.int64)
    for core in range(N_CORES):
        lo, hi = core * SLICE, (core + 1) * SLICE
        m = (dst >= lo) & (dst < hi)
        w_of = (dst[m] - lo) // P
        is_lo = src[m] < HALF
        cl = np.bincount(w_of[is_lo], minlength=NWIN)
        ch = np.bincount(w_of[~is_lo], minlength=NWIN)
        lowC = np.maximum(lowC, (cl + P - 1) // P)
        highC = np.maximum(highC, (ch + P - 1) // P)
    return tuple(int(x) for x in lowC), tuple(int(x) for x in highC)


def _prepare_core_inputs(core, src, dst, lowC, highC):
    import ml_dtypes
    cpw = [l + h for l, h in zip(lowC, highC)]
    C_TOT = sum(cpw)
    cbase = np.concatenate([[0], np.cumsum(cpw)]).astype(int)

    lo, hi = core * SLICE, (core + 1) * SLICE
    m = (dst >= lo) & (dst < hi)
    dsl = dst[m] - lo
    ssl = src[m]
    w_of = dsl // P
    is_lo = ssl < HALF

    slot_src = np.zeros((C_TOT, P), np.int64)
    slot_doff = np.full((C_TOT, P), -1.0, np.float32)

    # sort each (window, table-half) section by src so the gather reads the
    # table in ascending row order (HBM locality)
    order = np.lexsort((ssl, ~is_lo, w_of))
    ssl_o, dsl_o, w_o, lo_o = ssl[order], dsl[order], w_of[order], is_lo[order]
    for w in range(NWIN):
        wm = w_o == w
        for half, cb, g in ((True, cbase[w], lowC[w]),
                            (False, cbase[w] + lowC[w], highC[w])):
            sel = wm & (lo_o == half)
            s_w = ssl_o[sel]
            d_w = dsl_o[sel] - w * P
            n = len(s_w)
            assert n <= g * P, (core, w, half, n, g * P)
            flat_s = slot_src[cb:cb + g].reshape(-1)
            flat_d = slot_doff[cb:cb + g].reshape(-1)
            flat_s[:n] = s_w
            flat_d[:n] = d_w

    dstf = slot_doff.T.copy().astype(ml_dtypes.bfloat16)  # [P, C_TOT]

    ohg = np.zeros((P, C_TOT, P), ml_dtypes.float8_e4m3)
    cc, ee = np.nonzero(slot_doff >= 0)
    ohg[slot_doff[cc, ee].astype(np.int64), cc, ee] = 1

    idx_cols = []
    for w in range(NWIN):
        for half, cb, g in ((True, cbase[w], lowC[w]),
                            (False, cbase[w] + lowC[w], highC[w])):
            if not g:
                continue
            s = slot_src[cb:cb + g].reshape(-1).copy()
            if not half:
                s = np.maximum(s - HALF, 0)
            idx = s.astype(np.int16)
            idx_cols.append(np.tile(idx.reshape(-1, 16).T, (8, 1)))
    gidx = np.concatenate(idx_cols, axis=1).astype(np.int16)
    return dstf, gidx, ohg


def _shared_inputs(inputs):
    import ml_dtypes
    feat = np.asarray(inputs["feat"], np.float32)
    Wsrc = np.asarray(inputs["Wsrc"], np.float32)
    Wdst = np.asarray(inputs["Wdst"], np.float32)
    Wval = np.asarray(inputs["Wval"], np.float32)
    attn = np.asarray(inputs["attn"], np.float32).reshape(NUM_HEADS, FPH)

    featb = np.zeros((N_PAD, IN_FEATS), ml_dtypes.bfloat16)
    featb[:N_NODES] = feat.astype(ml_dtypes.bfloat16)
    # pre-transposed/tiled: [tile, f-partition, block, node]
    featTb = np.ascontiguousarray(
        featb.reshape(NTILES, P, 2, P).transpose(0, 3, 2, 1))

    # weights transposed + output-column permuted to (f,h)-major.
    # LayerNorm mean-fold: h@W^T = rstd*(feat@W'^T) with W' = W - rowmean(W)
    # (the -mu*ones part of LN folds into centered weight rows; rstd is
    # applied per-node after the GEMM).
    WsrcP = Wsrc[_OLD_OF_NEW, :]   # [256 newcol, 256 in]
    WvalP = Wval[_OLD_OF_NEW, :]
    WdstP = Wdst[_OLD_OF_NEW, :]
    WsrcP = WsrcP - WsrcP.mean(axis=1, keepdims=True)
    WvalP = WvalP - WvalP.mean(axis=1, keepdims=True)
    WdstP = WdstP - WdstP.mean(axis=1, keepdims=True)
    wfsv = np.zeros((P, 2, TBL_COLS), np.float32)
    for b in range(2):
        wfsv[:, b, 0:OUT_FEATS] = WsrcP[:, b * P:(b + 1) * P].T
        wfsv[:, b, OUT_FEATS:] = WvalP[:, b * P:(b + 1) * P].T
    wfd = np.zeros((P, 2, OUT_FEATS), np.float32)
    for b in range(2):
        wfd[:, b, :] = WdstP[:, b * P:(b + 1) * P].T

    attnb = np.zeros((P, 2, NUM_HEADS), np.float32)
    for b in range(2):
        j = b * P + np.arange(P)
        attnb[np.arange(P), b, j % NUM_HEADS] = attn[j % NUM_HEADS,
                                                     j // NUM_HEADS]

    identb = np.eye(P, dtype=ml_dtypes.bfloat16)
    iotab = np.tile(np.arange(P, dtype=ml_dtypes.bfloat16).reshape(1, P),
                    (P, 1))
    bf = ml_dtypes.bfloat16
    return (featTb, feat, wfsv.astype(bf), wfd.astype(bf), attnb.astype(bf),
            identb, iotab)


def make_in_maps(inputs, lowC, highC):
    import ml_dtypes
    featTb, feat, wfsv, wfd, attnb, identb, iotab = _shared_inputs(inputs)
    src = np.asarray(inputs["src"], np.int64)
    dst = np.asarray(inputs["dst"], np.int64)
    in_maps = []
    for core in range(N_CORES):
        dstf, gidx, ohg = _prepare_core_inputs(core, src, dst, lowC, highC)
        featmy = np.zeros((SLICE_PAD, IN_FEATS), ml_dtypes.bfloat16)
        n = min(SLICE_PAD, N_NODES - core * SLICE)
        featmy[:n] = feat[core * SLICE:core * SLICE + n].astype(
            ml_dtypes.bfloat16)
        featmyT = np.ascontiguousarray(
            featmy.reshape(NWIN, P, 2, P).transpose(0, 3, 2, 1))
        in_maps.append(dict(
            featTb=featTb, featmyT=featmyT, featmy=featmy, wfsv=wfsv, wfd=wfd,
            attnb=attnb, identb=identb, iotab=iotab, dstf=dstf, gidx=gidx,
            ohg=ohg,
        ))
    return in_maps


def kernel(**inputs):
    import concourse.bass_utils as bass_utils

    for b in ("bsrc", "bdst", "bval"):
        assert not np.any(np.asarray(inputs[b])), \
            "nonzero biases unsupported by this kernel"
    src = np.asarray(inputs["src"], np.int64)
    dst = np.asarray(inputs["dst"], np.int64)

    lowC, highC = _derive_schedule(src, dst)
    key = (lowC, highC)
    if key not in _CACHE:
        _CACHE[key] = _build_nc(lowC, highC)
    nc, _ = _CACHE[key]

    in_maps = make_in_maps(inputs, lowC, highC)
    res = bass_utils.run_bass_kernel_spmd(nc, in_maps, core_ids=list(range(N_CORES)))
    out = np.concatenate(
        [res.results[c]["outmy"][:SLICE] for c in range(N_CORES)], axis=0)
    return np.ascontiguousarray(out.astype(np.float32))



# revision 44
# speedup vs baseline: 1.7901x; 1.1131x over previous
"""GATv3Conv Trainium2 kernel (8 NeuronCores, SPMD).

Strategy (v4):
  - Shard EDGES by destination-node slice (core k owns dst in [k*6250,(k+1)*6250)).
    Segment softmax + aggregation are fully core-local (no collectives).
  - LayerNorm MEAN is folded into centered weight rows on host
    (h@W^T = rstd*(feat@W'^T), W' = W - rowmean(W)); rstd is applied to the
    GEMM output per node (DVE fs-half, ACT fv-half).
  - feat arrives HOST-pre-transposed/tiled ([tile, f-part, blk, node]) so
    phase A needs no natural-layout load and no on-device transpose; LN stats
    come from ACT Square + PE ones-matmuls (sum/sumsq in PSUM).
  - Each core redundantly computes the src/val GEMM table [N,512] = [fs|fv]
    bf16 in its HBM ((f,h)-major cols); dst GEMM only for its slice (SBUF).
  - Edge phase, per 128-dst-node window, edges in 128-edge chunks:
      * fsv rows gathered via gpsimd.dma_gather, round-robined over 4 SWDGE
        queues (single queue serializes the drain: 715us -> ~340us);
        4 G-pool bufs give the gather stream lookahead.
      * evT[f,e] = (one-hot dst gather of fd via matmul; one-hots in fp8) +
        (fs^T via identity matmul), accumulated in PSUM; silu on ACT.
      * score[e,h] via PE: lhsT=sevT half, rhs=block-diag attn [128,8].
      * ONE Exp per NPAR(3)-window group (shared score PSUM tile): silu and
        exp share no ACT table set, so each transition costs a 1.3us table
        load - batch them.
      * md[:,c,0:256] = fv*e8 (DVE bf16 2x broadcast); md[:,c,256:264] = e8 so
        a single one-hot agg matmul chain also produces denominators ([P,264]).
  - Softmax division deferred to batched phase C: out = silu(num/den + h),
    written bf16. exp() uses raw scores (no segment max, scores O(+-10)).
"""

import numpy as np

N_NODES = 50000
IN_FEATS = 256
OUT_FEATS = 256
NUM_HEADS = 8
FPH = OUT_FEATS // NUM_HEADS   # 32
LN_EPS = 1e-5
N_CORES = 8
SLICE = N_NODES // N_CORES     # 6250
P = 128
NWIN = (SLICE + P - 1) // P    # 49
SLICE_PAD = NWIN * P           # 6272
N_PAD = ((N_NODES + P - 1) // P) * P   # 50048
NTILES = N_PAD // P            # 391
HALF = 32768
TBL_COLS = 2 * OUT_FEATS       # 512
AGG_COLS = OUT_FEATS + NUM_HEADS  # 264
import os as _osm
ATILE = int(_osm.environ.get("ATILE", "4"))  # node tiles per phase-A DMA batch

# new feature order is (f, h)-major: new col j=f*8+h <- old col h*32+f
_OLD_OF_NEW = (np.arange(OUT_FEATS) % NUM_HEADS) * FPH + \
    np.arange(OUT_FEATS) // NUM_HEADS

_CACHE = {}


def _build_nc(lowC, highC, reps=1, phases="ABC"):
    import concourse.bacc as bacc
    import concourse.tile as tile
    from concourse import mybir
    from contextlib import ExitStack

    f32 = mybir.dt.float32
    bf16 = mybir.dt.bfloat16
    f8 = mybir.dt.float8e4
    i16 = mybir.dt.int16
    AF = mybir.ActivationFunctionType
    Alu = mybir.AluOpType

    lowC = list(lowC)
    highC = list(highC)
    cpw = [l + h for l, h in zip(lowC, highC)]
    cbase = np.concatenate([[0], np.cumsum(cpw)]).astype(int)
    C_TOT = int(cbase[-1])
    CPWMX = max(cpw)
    LCMX = max(lowC)
    HCMX = max(max(highC), 1)
    icols = 8 * C_TOT  # int16 idx cols (128 idx -> 8 cols of 16)

    import os as _os0
    GQN = int(_os0.environ.get("GQN", "4"))
    nc = bacc.Bacc(None, target_bir_lowering=False, num_swdge_queues=GQN)

    featTb_t = nc.dram_tensor("featTb", [NTILES, P, 2, P], bf16,
                              kind="ExternalInput")
    featmyT_t = nc.dram_tensor("featmyT", [NWIN, P, 2, P], bf16,
                               kind="ExternalInput")
    featmy_t = nc.dram_tensor("featmy", [SLICE_PAD, IN_FEATS], bf16,
                              kind="ExternalInput")
    wfsv_t = nc.dram_tensor("wfsv", [P, 2, TBL_COLS], bf16, kind="ExternalInput")
    wfd_t = nc.dram_tensor("wfd", [P, 2, OUT_FEATS], bf16, kind="ExternalInput")
    attnb_t = nc.dram_tensor("attnb", [P, 2, NUM_HEADS], bf16, kind="ExternalInput")
    identb_t = nc.dram_tensor("identb", [P, P], bf16, kind="ExternalInput")
    iotab_t = nc.dram_tensor("iotab", [P, P], bf16, kind="ExternalInput")
    dstf_t = nc.dram_tensor("dstf", [P, C_TOT], bf16, kind="ExternalInput")
    gidx_t = nc.dram_tensor("gidx", [P, icols], i16, kind="ExternalInput")
    ohg_t = nc.dram_tensor("ohg", [P, C_TOT, P], f8, kind="ExternalInput")
    out_t = nc.dram_tensor("outmy", [SLICE_PAD, OUT_FEATS], bf16,
                           kind="ExternalOutput")

    fsv_t = nc.dram_tensor("fsvtbl", [N_PAD, TBL_COLS], bf16, kind="Internal")
    hupd_t = nc.dram_tensor("hupdtbl", [NWIN, P, AGG_COLS], bf16, kind="Internal")

    with tile.TileContext(nc) as tc, ExitStack() as ctx:
        if reps > 1:
            ctx.enter_context(tc.For_i(0, reps, 1))
        const = ctx.enter_context(tc.tile_pool(name="const", bufs=1))
        persist = ctx.enter_context(tc.tile_pool(name="persist", bufs=1))

        wfsv = const.tile([P, 2, TBL_COLS], bf16)
        nc.sync.dma_start(out=wfsv, in_=wfsv_t[:, :, :])
        wfd = const.tile([P, 2, OUT_FEATS], bf16)
        nc.sync.dma_start(out=wfd, in_=wfd_t[:, :, :])
        attnb = const.tile([P, 2, NUM_HEADS], bf16)
        nc.sync.dma_start(out=attnb, in_=attnb_t[:, :, :])
        identb = const.tile([P, P], bf16)
        nc.sync.dma_start(out=identb, in_=identb_t[:, :])
        iotab = const.tile([P, P], bf16)
        nc.sync.dma_start(out=iotab, in_=iotab_t[:, :])
        dstf = const.tile([P, C_TOT], bf16)
        nc.sync.dma_start(out=dstf, in_=dstf_t[:, :])
        gidx = const.tile([P, icols], i16)
        nc.sync.dma_start(out=gidx, in_=gidx_t[:, :])
        eps_c = const.tile([P, 1], f32)
        nc.vector.memset(eps_c[:], LN_EPS)
        ones_c = const.tile([P, 1], bf16)
        nc.vector.memset(ones_c[:], 1.0)

        fd_slice = persist.tile([P, NWIN, OUT_FEATS], bf16)
        stats_my = persist.tile([P, NWIN, 2], f32)   # (mean, rstd)
        husum = persist.tile([P, NWIN, AGG_COLS], bf16)  # per-window aggregates

        # ---------------- Phase A: LN + GEMM tables ----------------
        # feat arrives pre-transposed/tiled from host ([tile, f-part, blk, node]).
        # LN mean is folded into centered weights; stats (sum/sumsq) come from
        # ones-matmuls on hT and ACT Square, so no natural-layout load at all.
        def node_sweep(srcT_dram, ntiles, emit, wtile, wcols):
            import os as _osA
            with tc.tile_pool(name="a_sb", bufs=int(_osA.environ.get("ASB", "4"))) as sbp, \
                 tc.tile_pool(name="a_ps", bufs=int(_osA.environ.get("APS", "4")), space="PSUM") as psp, \
                 tc.tile_pool(name="a_st", bufs=int(_osA.environ.get("AST", "3")), space="PSUM") as stp:
                for t0 in range(0, ntiles, ATILE):
                    bt = min(ATILE, ntiles - t0)
                    hT4 = sbp.tile([P, ATILE, 2, P], bf16, tag="hT4")
                    nc.sync.dma_start(
                        out=hT4[:, 0:bt, :, :],
                        in_=srcT_dram[t0:t0 + bt].rearrange("t p b e -> p t b e"))
                    sq4 = sbp.tile([P, ATILE, 2, P], bf16, tag="sq4")
                    nc.scalar.activation(out=sq4[:, 0:bt], in_=hT4[:, 0:bt],
                                         func=AF.Square)
                    sps = stp.tile([P, ATILE, 2], f32, tag="sps")
                    for i in range(bt):
                        for b in range(2):
                            nc.tensor.matmul(out=sps[:, i, 0:1],
                                             lhsT=hT4[:, i, b, :], rhs=ones_c[:],
                                             start=(b == 0), stop=(b == 1))
                        for b in range(2):
                            nc.tensor.matmul(out=sps[:, i, 1:2],
                                             lhsT=sq4[:, i, b, :], rhs=ones_c[:],
                                             start=(b == 0), stop=(b == 1))
                    # var = sumsq/256 - (sum/256)^2 ; rstd = 1/sqrt(var+eps)
                    mean4 = sbp.tile([P, ATILE], f32, tag="mean4")
                    nc.vector.tensor_scalar(out=mean4[:, 0:bt],
                                            in0=sps[:, 0:bt, 0],
                                            scalar1=1.0 / IN_FEATS, scalar2=None,
                                            op0=Alu.mult)
                    mu2 = sbp.tile([P, ATILE], f32, tag="mu2")
                    nc.scalar.activation(out=mu2[:, 0:bt], in_=sps[:, 0:bt, 0],
                                         func=AF.Square, scale=1.0 / IN_FEATS)
                    var4 = sbp.tile([P, ATILE], f32, tag="var4")
                    nc.vector.tensor_scalar(out=var4[:, 0:bt],
                                            in0=sps[:, 0:bt, 1],
                                            scalar1=1.0 / IN_FEATS,
                                            scalar2=None, op0=Alu.mult)
                    vc4 = sbp.tile([P, ATILE], f32, tag="vc4")
                    nc.vector.tensor_tensor(out=vc4[:, 0:bt], in0=var4[:, 0:bt],
                                            in1=mu2[:, 0:bt], op=Alu.subtract)
                    sd4 = sbp.tile([P, ATILE], f32, tag="sd4")
                    nc.scalar.activation(out=sd4[:, 0:bt], in_=vc4[:, 0:bt],
                                         func=AF.Sqrt, bias=eps_c[:])
                    rstd4 = sbp.tile([P, ATILE], f32, tag="rstd4")
                    nc.vector.reciprocal(out=rstd4[:, 0:bt], in_=sd4[:, 0:bt])
                    batch = []
                    for i in range(bt):
                        g = psp.tile([P, wcols], f32, tag="gemm")
                        nc.tensor.matmul(out=g[:], lhsT=hT4[:, i, 0, :],
                                         rhs=wtile[:, 0, :], start=True, stop=False)
                        nc.tensor.matmul(out=g[:], lhsT=hT4[:, i, 1, :],
                                         rhs=wtile[:, 1, :], start=False, stop=True)
                        emit(t0 + i, i, g, mean4, rstd4, sbp, batch)
                    if batch:
                        st4, bt0 = batch[0]
                        nc.sync.dma_start(
                            out=fsv_t[bt0 * P:(bt0 + bt) * P, :].rearrange(
                                "(t p) f -> p t f", p=P),
                            in_=st4[:, 0:bt, :])

        def emit_fsv(t, i, g, mean4, rstd4, sbp, batch):
            if i == 0:
                st4 = sbp.tile([P, ATILE, TBL_COLS], bf16, tag="fsv4")
                batch.append((st4, t))
            st4, _ = batch[0]
            nc.vector.tensor_scalar(
                out=st4[:, i, 0:OUT_FEATS], in0=g[:, 0:OUT_FEATS],
                scalar1=rstd4[:, i:i + 1], scalar2=None, op0=Alu.mult)
            nc.scalar.activation(out=st4[:, i, OUT_FEATS:], in_=g[:, OUT_FEATS:],
                                 func=AF.Copy, scale=rstd4[:, i:i + 1])

        def emit_fd(t, i, g, mean4, rstd4, sbp, batch):
            nc.vector.tensor_scalar(
                out=fd_slice[:, t, 0:P], in0=g[:, 0:P],
                scalar1=rstd4[:, i:i + 1], scalar2=None, op0=Alu.mult)
            nc.scalar.activation(out=fd_slice[:, t, P:OUT_FEATS], in_=g[:, P:],
                                 func=AF.Copy, scale=rstd4[:, i:i + 1])
            nc.vector.tensor_copy(out=stats_my[:, t, 0:1], in_=mean4[:, i:i + 1])
            nc.vector.tensor_copy(out=stats_my[:, t, 1:2],
                                  in_=rstd4[:, i:i + 1])

        node_sweep(featmyT_t, NWIN, emit_fd, wfd, OUT_FEATS)
        if "A" in phases:
            node_sweep(featTb_t, NTILES, emit_fsv, wfsv, TBL_COLS)

        # ---------------- Phase B: edge phase ----------------
        fsv_hi = fsv_t[HALF:N_PAD, :]
        if "B" not in phases:
            NWIN_B = 0
        else:
            NWIN_B = NWIN
        import os as _os
        _bb = lambda k, d: int(_os.environ.get(k, d))
        with tc.tile_pool(name="b_glo", bufs=_bb("BGL", 4)) as glop, \
             tc.tile_pool(name="b_ghi", bufs=_bb("BGH", 4)) as ghip, \
             tc.tile_pool(name="b_ohg", bufs=_bb("BOG", 3)) as ohgp, \
             tc.tile_pool(name="b_oha", bufs=_bb("BOA", 3)) as ohap, \
             tc.tile_pool(name="b_sev", bufs=_bb("BSV", 4)) as sevp, \
             tc.tile_pool(name="b_e8", bufs=_bb("BE8", 2)) as e8p, \
             tc.tile_pool(name="b_md", bufs=_bb("BMD", 3)) as mdp, \
             tc.tile_pool(name="b_hw", bufs=_bb("BHW", 3)) as hwp, \
             tc.tile_pool(name="b_ev", bufs=_bb("BEV", 2), space="PSUM") as evp, \
             tc.tile_pool(name="b_sc", bufs=_bb("BSC", 2), space="PSUM") as scp, \
             tc.tile_pool(name="b_hu", bufs=_bb("BHU", 2), space="PSUM") as hup:

            icol_of = np.concatenate([[0], np.cumsum([8 * c for c in cpw])])
            _qctr = [0]

            def _next_q():
                q = _qctr[0] % GQN
                _qctr[0] += 1
                return q

            def win_gather(w):
                """Issue gathers + oh loads + one-hot builds for window w."""
                cb = cbase[w]
                icol = int(icol_of[w])
                tiles = {}
                ohg_w = ohgp.tile([P, CPWMX, P], f8, tag="ohg")
                if LVL >= 2:
                    nc.sync.dma_start(out=ohg_w[:, 0:cpw[w], :],
                                      in_=ohg_t[:, cb:cb + cpw[w], :])
                import os as _os2
                _SEQDMA = _os2.environ.get("DBG_SEQDMA") == "1"
                # >1024 idx per dma_gather call wedges the device
                MAXC = int(_os2.environ.get("MAXC", "8"))
                if lowC[w] and _SEQDMA:
                    G = glop.tile([P, LCMX, TBL_COLS], bf16, tag="Glo")
                    nc.sync.dma_start(
                        out=G[:, 0:lowC[w], :],
                        in_=fsv_t[0:lowC[w] * P, :].rearrange(
                            "(t p) f -> p t f", p=P))
                    tiles["lo"] = G
                    icol += 8 * lowC[w]
                if highC[w] and _SEQDMA:
                    G = ghip.tile([P, HCMX, TBL_COLS], bf16, tag="Ghi")
                    nc.sync.dma_start(
                        out=G[:, 0:highC[w], :],
                        in_=fsv_t[0:highC[w] * P, :].rearrange(
                            "(t p) f -> p t f", p=P))
                    tiles["hi"] = G
                    icol += 8 * highC[w]
                if lowC[w] and not _SEQDMA:
                    G = glop.tile([P, LCMX, TBL_COLS], bf16, tag="Glo")
                    o = 0
                    while o < (lowC[w] if LVL >= 1 else 0):
                        g = min(MAXC, lowC[w] - o)
                        ni = g * P
                        nc.gpsimd.dma_gather(
                            out_ap=G[:, o:o + g, :], in_ap=fsv_t[:, :],
                            idxs_ap=gidx[:, icol:icol + ni // 16],
                            num_idxs=ni, num_idxs_reg=ni, elem_size=TBL_COLS,
                            queue_num=_next_q())
                        icol += ni // 16
                        o += g
                    tiles["lo"] = G
                if highC[w] and not _SEQDMA:
                    G = ghip.tile([P, HCMX, TBL_COLS], bf16, tag="Ghi")
                    o = 0
                    while o < (highC[w] if LVL >= 1 else 0):
                        g = min(MAXC, highC[w] - o)
                        ni = g * P
                        nc.gpsimd.dma_gather(
                            out_ap=G[:, o:o + g, :], in_ap=fsv_hi,
                            idxs_ap=gidx[:, icol:icol + ni // 16],
                            num_idxs=ni, num_idxs_reg=ni, elem_size=TBL_COLS,
                            queue_num=_next_q())
                        icol += ni // 16
                        o += g
                    tiles["hi"] = G
                oha_w = ohap.tile([P, CPWMX, P], bf16, tag="oha")
                if LVL >= 2:
                    nc.vector.tensor_tensor(
                        out=oha_w[:, 0:cpw[w], :],
                        in0=iotab[:, None, :].to_broadcast([P, cpw[w], P]),
                        in1=dstf[:, cb:cb + cpw[w], None].to_broadcast(
                            [P, cpw[w], P]),
                        op=Alu.is_equal)
                return tiles, ohg_w, oha_w

            def chunk_of(w, c):
                """(section G-key, local idx) for chunk c of window w."""
                if c < lowC[w]:
                    return "lo", c
                return "hi", c - lowC[w]

            SG = int(_os.environ.get("BSG", "4"))  # chunks per silu group
            NPAR = int(_os.environ.get("BNP", "3"))  # windows in flight
            assert NPAR * CPWMX * NUM_HEADS * 4 <= 2048, "score tile > PSUM bank"

            def score_group(w, wi, tiles, ohg_w, score_ps, c0):
                gn = min(SG, cpw[w] - c0)
                ev2 = evp.tile([P, 2, SG, P], f32, tag="ev2")
                if LVL >= 3:
                    for i in range(gn):
                        sec, lc = chunk_of(w, c0 + i)
                        G = tiles[sec]
                        for b in range(2):
                            nc.tensor.matmul(
                                out=ev2[:, b, i, :],
                                lhsT=fd_slice[:, w, b * P:(b + 1) * P],
                                rhs=ohg_w[:, c0 + i, :], start=True, stop=False)
                            nc.tensor.matmul(
                                out=ev2[:, b, i, :],
                                lhsT=G[:, lc, b * P:(b + 1) * P],
                                rhs=identb[:], start=False, stop=True)
                sevT = sevp.tile([P, 2, SG, P], bf16, tag="sevT")
                if LVL >= 4:
                    nc.scalar.activation(out=sevT[:, :, 0:gn, :],
                                         in_=ev2[:, :, 0:gn, :], func=AF.Silu)
                for i in range(gn if LVL >= 5 else 0):
                    for b in range(2):
                        nc.tensor.matmul(
                            out=score_ps[:, wi, c0 + i, :],
                            lhsT=sevT[:, b, i, :], rhs=attnb[:, b, :],
                            start=(b == 0), stop=(b == 1))

            def win_finish(w, wi, tiles, ohg_w, oha_w, e8g):
                # md[:, c, 0:256] = fv * exp(score) broadcast; md[:, c, 256:264]
                # = exp(score) so ONE agg matmul also produces denominators.
                md = mdp.tile([P, CPWMX, AGG_COLS], bf16, tag="md")
                if LVL >= 5:
                    nc.vector.tensor_copy(out=md[:, 0:cpw[w], OUT_FEATS:],
                                          in_=e8g[:, wi, 0:cpw[w], :])
                for sec, g, off in (("lo", lowC[w], 0), ("hi", highC[w], lowC[w])):
                    if not g or LVL < 6:
                        continue
                    G = tiles[sec]
                    nc.vector.tensor_tensor(
                        out=md[:, off:off + g, 0:OUT_FEATS].rearrange(
                            "p c (f h) -> p c f h", h=NUM_HEADS),
                        in0=G[:, 0:g, OUT_FEATS:].rearrange(
                            "p c (f h) -> p c f h", h=NUM_HEADS),
                        in1=md[:, off:off + g, None, OUT_FEATS:].to_broadcast(
                            [P, g, FPH, NUM_HEADS]),
                        op=Alu.mult)
                # note: matmul accumulation groups must stay contiguous per
                # PSUM region on HW - interleaving two regions corrupts one.
                if LVL < 7:
                    return
                hupd = hup.tile([P, AGG_COLS], f32, tag="hupd")
                for c in range(cpw[w]):
                    nc.tensor.matmul(out=hupd[:, 0:AGG_COLS],
                                     lhsT=oha_w[:, c, :], rhs=md[:, c, :],
                                     start=(c == 0), stop=(c == cpw[w] - 1))
                nc.vector.tensor_copy(out=husum[:, w, :], in_=hupd[:])

            LVL = int(_os.environ.get("DBG_LEVEL", "99"))
            w = 0
            while w < NWIN_B:
                grp = list(range(w, min(w + NPAR, NWIN_B)))
                nw = len(grp)
                score_ps = scp.tile([P, NPAR, CPWMX, NUM_HEADS], f32,
                                    tag="score")
                state = []
                for wi, ww in enumerate(grp):
                    tiles, ohg_w, oha_w = win_gather(ww)
                    state.append((ww, wi, tiles, ohg_w, oha_w))
                maxg = max((cpw[ww] + SG - 1) // SG for ww in grp)
                for gi in range(maxg):
                    for (ww, wi, tiles, ohg_w, oha_w) in state:
                        if gi * SG < cpw[ww]:
                            score_group(ww, wi, tiles, ohg_w, score_ps, gi * SG)
                # one exp per window-group: silu<->exp share no ACT table set,
                # so each transition costs a 1.3us table load - batch them.
                e8g = e8p.tile([P, NPAR, CPWMX, NUM_HEADS], bf16, tag="e8g")
                if LVL >= 5:
                    nc.scalar.activation(out=e8g[:, 0:nw], in_=score_ps[:, 0:nw],
                                         func=AF.Exp)
                for (ww, wi, tiles, ohg_w, oha_w) in state:
                    win_finish(ww, wi, tiles, ohg_w, oha_w, e8g)
                w += len(grp)

        # ---------------- Phase C: normalize + residual + silu ----------------
        with tc.tile_pool(name="c_sb", bufs=3) as sbp:
            for w0 in range(0, NWIN if "C" in phases else 0, ATILE):
                bt = min(ATILE, NWIN - w0)
                hu4 = husum[:, w0:w0 + bt, :]
                F4 = sbp.tile([P, ATILE, IN_FEATS], bf16, tag="F4")
                nc.sync.dma_start(
                    out=F4[:, 0:bt, :],
                    in_=featmy_t[w0 * P:(w0 + bt) * P, :].rearrange(
                        "(t p) f -> p t f", p=P))
                h4 = sbp.tile([P, ATILE, IN_FEATS], bf16, tag="h")
                for i in range(bt):
                    w = w0 + i
                    nc.vector.tensor_scalar(
                        out=h4[:, i, :], in0=F4[:, i, :],
                        scalar1=stats_my[:, w, 0:1],
                        scalar2=stats_my[:, w, 1:2],
                        op0=Alu.subtract, op1=Alu.mult)
                den = sbp.tile([P, ATILE, NUM_HEADS], f32, tag="den")
                nc.vector.tensor_scalar_add(out=den[:, 0:bt],
                                            in0=hu4[:, 0:bt, OUT_FEATS:],
                                            scalar1=1e-30)
                denr = sbp.tile([P, ATILE, NUM_HEADS], f32, tag="denr")
                nc.vector.reciprocal(out=denr[:, 0:bt], in_=den[:, 0:bt])
                o = sbp.tile([P, ATILE, FPH, NUM_HEADS], f32, tag="o")
                nc.vector.tensor_tensor(
                    out=o[:, 0:bt],
                    in0=hu4[:, 0:bt, 0:OUT_FEATS].rearrange(
                        "p w (f h) -> p w f h", h=NUM_HEADS),
                    in1=denr[:, 0:bt, None, :].to_broadcast(
                        [P, bt, FPH, NUM_HEADS]),
                    op=Alu.mult)
                on = sbp.tile([P, ATILE, OUT_FEATS], f32, tag="on")
                nc.vector.tensor_tensor(
                    out=on[:, 0:bt].rearrange("p w (h f) -> p w h f",
                                              h=NUM_HEADS),
                    in0=o[:, 0:bt].rearrange("p w f h -> p w h f"),
                    in1=h4[:, 0:bt].rearrange("p w (h f) -> p w h f",
                                              h=NUM_HEADS),
                    op=Alu.add)
                oo = sbp.tile([P, ATILE, OUT_FEATS], bf16, tag="oo")
                nc.scalar.activation(out=oo[:, 0:bt], in_=on[:, 0:bt],
                                     func=AF.Silu)
                nc.sync.dma_start(
                    out=out_t[w0 * P:(w0 + bt) * P, :].rearrange(
                        "(w p) f -> p w f", p=P),
                    in_=oo[:, 0:bt])

    nc.compile()
    return nc, (lowC, highC)


def _derive_schedule(src, dst):
    """Per-window chunk counts (low/high table half), maxed over cores."""
    lowC = np.zeros(NWIN, np.int64)
    highC = np.zeros(NWIN, np.int64)
    for core in range(N_CORES):
        lo, hi = core * SLICE, (core + 1) * SLICE
        m = (dst >= lo) & (dst < hi)
        w_of = (dst[m] - lo) // P
        is_lo = src[m] < HALF
        cl = np.bincount(w_of[is_lo], minlength=NWIN)
        ch = np.bincount(w_of[~is_lo], minlength=NWIN)
        lowC = np.maximum(lowC, (cl + P - 1) // P)
        highC = np.maximum(highC, (ch + P - 1) // P)
    return tuple(int(x) for x in lowC), tuple(int(x) for x in highC)


def _prepare_core_inputs(core, src, dst, lowC, highC):
    import ml_dtypes
    cpw = [l + h for l, h in zip(lowC, highC)]
    C_TOT = sum(cpw)
    cbase = np.concatenate([[0], np.cumsum(cpw)]).astype(int)

    lo, hi = core * SLICE, (core + 1) * SLICE
    m = (dst >= lo) & (dst < hi)
    dsl = dst[m] - lo
    ssl = src[m]
    w_of = dsl // P
    is_lo = ssl < HALF

    slot_src = np.zeros((C_TOT, P), np.int64)
    slot_doff = np.full((C_TOT, P), -1.0, np.float32)

    # sort each (window, table-half) section by src so the gather reads the
    # table in ascending row order (HBM locality)
    order = np.lexsort((ssl, ~is_lo, w_of))
    ssl_o, dsl_o, w_o, lo_o = ssl[order], dsl[order], w_of[order], is_lo[order]
    for w in range(NWIN):
        wm = w_o == w
        for half, cb, g in ((True, cbase[w], lowC[w]),
                            (False, cbase[w] + lowC[w], highC[w])):
            sel = wm & (lo_o == half)
            s_w = ssl_o[sel]
            d_w = dsl_o[sel] - w * P
            n = len(s_w)
            assert n <= g * P, (core, w, half, n, g * P)
            flat_s = slot_src[cb:cb + g].reshape(-1)
            flat_d = slot_doff[cb:cb + g].reshape(-1)
            flat_s[:n] = s_w
            flat_d[:n] = d_w

    dstf = slot_doff.T.copy().astype(ml_dtypes.bfloat16)  # [P, C_TOT]

    ohg = np.zeros((P, C_TOT, P), ml_dtypes.float8_e4m3)
    cc, ee = np.nonzero(slot_doff >= 0)
    ohg[slot_doff[cc, ee].astype(np.int64), cc, ee] = 1

    idx_cols = []
    for w in range(NWIN):
        for half, cb, g in ((True, cbase[w], lowC[w]),
                            (False, cbase[w] + lowC[w], highC[w])):
            if not g:
                continue
            s = slot_src[cb:cb + g].reshape(-1).copy()
            if not half:
                s = np.maximum(s - HALF, 0)
            idx = s.astype(np.int16)
            idx_cols.append(np.tile(idx.reshape(-1, 16).T, (8, 1)))
    gidx = np.concatenate(idx_cols, axis=1).astype(np.int16)
    return dstf, gidx, ohg


def _shared_inputs(inputs):
    import ml_dtypes
    feat = np.asarray(inputs["feat"], np.float32)
    Wsrc = np.asarray(inputs["Wsrc"], np.float32)
    Wdst = np.asarray(inputs["Wdst"], np.float32)
    Wval = np.asarray(inputs["Wval"], np.float32)
    attn = np.asarray(inputs["attn"], np.float32).reshape(NUM_HEADS, FPH)

    featb = np.zeros((N_PAD, IN_FEATS), ml_dtypes.bfloat16)
    featb[:N_NODES] = feat.astype(ml_dtypes.bfloat16)
    # pre-transposed/tiled: [tile, f-partition, block, node]
    featTb = np.ascontiguousarray(
        featb.reshape(NTILES, P, 2, P).transpose(0, 3, 2, 1))

    # weights transposed + output-column permuted to (f,h)-major.
    # LayerNorm mean-fold: h@W^T = rstd*(feat@W'^T) with W' = W - rowmean(W)
    # (the -mu*ones part of LN folds into centered weight rows; rstd is
    # applied per-node after the GEMM).
    WsrcP = Wsrc[_OLD_OF_NEW, :]   # [256 newcol, 256 in]
    WvalP = Wval[_OLD_OF_NEW, :]
    WdstP = Wdst[_OLD_OF_NEW, :]
    WsrcP = WsrcP - WsrcP.mean(axis=1, keepdims=True)
    WvalP = WvalP - WvalP.mean(axis=1, keepdims=True)
    WdstP = WdstP - WdstP.mean(axis=1, keepdims=True)
    wfsv = np.zeros((P, 2, TBL_COLS), np.float32)
    for b in range(2):
        wfsv[:, b, 0:OUT_FEATS] = WsrcP[:, b * P:(b + 1) * P].T
        wfsv[:, b, OUT_FEATS:] = WvalP[:, b * P:(b + 1) * P].T
    wfd = np.zeros((P, 2, OUT_FEATS), np.float32)
    for b in range(2):
        wfd[:, b, :] = WdstP[:, b * P:(b + 1) * P].T

    attnb = np.zeros((P, 2, NUM_HEADS), np.float32)
    for b in range(2):
        j = b * P + np.arange(P)
        attnb[np.arange(P), b, j % NUM_HEADS] = attn[j % NUM_HEADS,
                                                     j // NUM_HEADS]

    identb = np.eye(P, dtype=ml_dtypes.bfloat16)
    iotab = np.tile(np.arange(P, dtype=ml_dtypes.bfloat16).reshape(1, P),
                    (P, 1))
    bf = ml_dtypes.bfloat16
    return (featTb, feat, wfsv.astype(bf), wfd.astype(bf), attnb.astype(bf),
            identb, iotab)


def make_in_maps(inputs, lowC, highC):
    import ml_dtypes
    featTb, feat, wfsv, wfd, attnb, identb, iotab = _shared_inputs(inputs)
    src = np.asarray(inputs["src"], np.int64)
    dst = np.asarray(inputs["dst"], np.int64)
    in_maps = []
    for core in range(N_CORES):
        dstf, gidx, ohg = _prepare_core_inputs(core, src, dst, lowC, highC)
        featmy = np.zeros((SLICE_PAD, IN_FEATS), ml_dtypes.bfloat16)
        n = min(SLICE_PAD, N_NODES - core * SLICE)
        featmy[:n] = feat[core * SLICE:core * SLICE + n].astype(
            ml_dtypes.bfloat16)
        featmyT = np.ascontiguousarray(
            featmy.reshape(NWIN, P, 2, P).transpose(0, 3, 2, 1))
        in_maps.append(dict(
            featTb=featTb, featmyT=featmyT, featmy=featmy, wfsv=wfsv, wfd=wfd,
            attnb=attnb, identb=identb, iotab=iotab, dstf=dstf, gidx=gidx,
            ohg=ohg,
        ))
    return in_maps


def kernel(**inputs):
    import concourse.bass_utils as bass_utils

    for b in ("bsrc", "bdst", "bval"):
        assert not np.any(np.asarray(inputs[b])), \
            "nonzero biases unsupported by this kernel"
    src = np.asarray(inputs["src"], np.int64)
    dst = np.asarray(inputs["dst"], np.int64)

    lowC, highC = _derive_schedule(src, dst)
    key = (lowC, highC)
    if key not in _CACHE:
        _CACHE[key] = _build_nc(lowC, highC)
    nc, _ = _CACHE[key]

    in_maps = make_in_maps(inputs, lowC, highC)
    res = bass_utils.run_bass_kernel_spmd(nc, in_maps, core_ids=list(range(N_CORES)))
    out = np.concatenate(
        [res.results[c]["outmy"][:SLICE] for c in range(N_CORES)], axis=0)
    return np.ascontiguousarray(out.astype(np.float32))



# revision 45
# speedup vs baseline: 1.9340x; 1.0804x over previous
"""GATv3Conv Trainium2 kernel (8 NeuronCores, SPMD).

Strategy (v4):
  - Shard EDGES by destination-node slice (core k owns dst in [k*6250,(k+1)*6250)).
    Segment softmax + aggregation are fully core-local (no collectives).
  - LayerNorm MEAN is folded into centered weight rows on host
    (h@W^T = rstd*(feat@W'^T), W' = W - rowmean(W)); rstd is applied to the
    GEMM output per node (DVE fs-half, ACT fv-half).
  - feat arrives HOST-pre-transposed/tiled ([tile, f-part, blk, node]) so
    phase A needs no natural-layout load and no on-device transpose; LN stats
    come from ACT Square + PE ones-matmuls (sum/sumsq in PSUM).
  - Each core redundantly computes the src/val GEMM table [N,512] = [fs|fv]
    bf16 in its HBM ((f,h)-major cols); dst GEMM only for its slice (SBUF).
  - Edge phase, per 128-dst-node window, edges in 128-edge chunks:
      * fsv rows gathered via gpsimd.dma_gather, round-robined over 4 SWDGE
        queues (single queue serializes the drain: 715us -> ~340us);
        4 G-pool bufs give the gather stream lookahead.
      * evT[f,e] = (one-hot dst gather of fd via matmul; one-hots in fp8) +
        (fs^T via identity matmul), accumulated in PSUM; silu on ACT.
      * score[e,h] via PE: lhsT=sevT half, rhs=block-diag attn [128,8].
      * ONE Exp per NPAR(3)-window group (shared score PSUM tile): silu and
        exp share no ACT table set, so each transition costs a 1.3us table
        load - batch them.
      * md[:,c,0:256] = fv*e8 (DVE bf16 2x broadcast); md[:,c,256:264] = e8 so
        a single one-hot agg matmul chain also produces denominators ([P,264]).
  - Softmax division deferred to batched phase C: out = silu(num/den + h),
    written bf16. exp() uses raw scores (no segment max, scores O(+-10)).
"""

import numpy as np

N_NODES = 50000
IN_FEATS = 256
OUT_FEATS = 256
NUM_HEADS = 8
FPH = OUT_FEATS // NUM_HEADS   # 32
LN_EPS = 1e-5
N_CORES = 8
SLICE = N_NODES // N_CORES     # 6250
P = 128
NWIN = (SLICE + P - 1) // P    # 49
SLICE_PAD = NWIN * P           # 6272
N_PAD = ((N_NODES + P - 1) // P) * P   # 50048
NTILES = N_PAD // P            # 391
HALF = 32768
TBL_COLS = 2 * OUT_FEATS       # 512
AGG_COLS = OUT_FEATS + NUM_HEADS  # 264
import os as _osm
ATILE = int(_osm.environ.get("ATILE", "4"))  # node tiles per phase-A DMA batch

# new feature order is (f, h)-major: new col j=f*8+h <- old col h*32+f
_OLD_OF_NEW = (np.arange(OUT_FEATS) % NUM_HEADS) * FPH + \
    np.arange(OUT_FEATS) // NUM_HEADS

_CACHE = {}


def _build_nc(lowC, highC, reps=1, phases="ABC"):
    import concourse.bacc as bacc
    import concourse.tile as tile
    from concourse import mybir
    from contextlib import ExitStack

    f32 = mybir.dt.float32
    bf16 = mybir.dt.bfloat16
    f8 = mybir.dt.float8e4
    i16 = mybir.dt.int16
    AF = mybir.ActivationFunctionType
    Alu = mybir.AluOpType

    lowC = list(lowC)
    highC = list(highC)
    cpw = [l + h for l, h in zip(lowC, highC)]
    cbase = np.concatenate([[0], np.cumsum(cpw)]).astype(int)
    C_TOT = int(cbase[-1])
    CPWMX = max(cpw)
    LCMX = max(lowC)
    HCMX = max(max(highC), 1)
    icols = 8 * C_TOT  # int16 idx cols (128 idx -> 8 cols of 16)

    import os as _os0
    GQN = int(_os0.environ.get("GQN", "4"))
    nc = bacc.Bacc(None, target_bir_lowering=False, num_swdge_queues=GQN)

    featTb_t = nc.dram_tensor("featTb", [NTILES, P, 2, P], bf16,
                              kind="ExternalInput")
    featmyT_t = nc.dram_tensor("featmyT", [NWIN, P, 2, P], bf16,
                               kind="ExternalInput")
    featmy_t = nc.dram_tensor("featmy", [SLICE_PAD, IN_FEATS], bf16,
                              kind="ExternalInput")
    wfsv_t = nc.dram_tensor("wfsv", [P, 2, TBL_COLS], bf16, kind="ExternalInput")
    wfd_t = nc.dram_tensor("wfd", [P, 2, OUT_FEATS], bf16, kind="ExternalInput")
    attnb_t = nc.dram_tensor("attnb", [P, 2, NUM_HEADS], bf16, kind="ExternalInput")
    identb_t = nc.dram_tensor("identb", [P, P], bf16, kind="ExternalInput")
    iotab_t = nc.dram_tensor("iotab", [P, P], bf16, kind="ExternalInput")
    dstf_t = nc.dram_tensor("dstf", [P, C_TOT], bf16, kind="ExternalInput")
    gidx_t = nc.dram_tensor("gidx", [P, icols], i16, kind="ExternalInput")
    ohg_t = nc.dram_tensor("ohg", [P, C_TOT, P], f8, kind="ExternalInput")
    out_t = nc.dram_tensor("outmy", [SLICE_PAD, OUT_FEATS], bf16,
                           kind="ExternalOutput")

    fsv_t = nc.dram_tensor("fsvtbl", [N_PAD, TBL_COLS], bf16, kind="Internal")
    hupd_t = nc.dram_tensor("hupdtbl", [NWIN, P, AGG_COLS], bf16, kind="Internal")

    with tile.TileContext(nc) as tc, ExitStack() as ctx:
        if reps > 1:
            ctx.enter_context(tc.For_i(0, reps, 1))
        const = ctx.enter_context(tc.tile_pool(name="const", bufs=1))
        persist = ctx.enter_context(tc.tile_pool(name="persist", bufs=1))

        wfsv = const.tile([P, 2, TBL_COLS], bf16)
        nc.sync.dma_start(out=wfsv, in_=wfsv_t[:, :, :])
        wfd = const.tile([P, 2, OUT_FEATS], bf16)
        nc.sync.dma_start(out=wfd, in_=wfd_t[:, :, :])
        attnb = const.tile([P, 2, NUM_HEADS], bf16)
        nc.sync.dma_start(out=attnb, in_=attnb_t[:, :, :])
        identb = const.tile([P, P], bf16)
        nc.sync.dma_start(out=identb, in_=identb_t[:, :])
        iotab = const.tile([P, P], bf16)
        nc.sync.dma_start(out=iotab, in_=iotab_t[:, :])
        dstf = const.tile([P, C_TOT], bf16)
        nc.sync.dma_start(out=dstf, in_=dstf_t[:, :])
        gidx = const.tile([P, icols], i16)
        nc.sync.dma_start(out=gidx, in_=gidx_t[:, :])
        eps_c = const.tile([P, 1], f32)
        nc.vector.memset(eps_c[:], LN_EPS)
        ones_c = const.tile([P, 1], bf16)
        nc.vector.memset(ones_c[:], 1.0)

        fd_slice = persist.tile([P, NWIN, OUT_FEATS], bf16)
        stats_my = persist.tile([P, NWIN, 2], f32)   # (mean, rstd)
        husum = persist.tile([P, NWIN, AGG_COLS], bf16)  # per-window aggregates

        # ---------------- Phase A: LN + GEMM tables ----------------
        # feat arrives pre-transposed/tiled from host ([tile, f-part, blk, node]).
        # LN mean is folded into centered weights; stats (sum/sumsq) come from
        # ones-matmuls on hT and ACT Square, so no natural-layout load at all.
        def node_sweep(srcT_dram, ntiles, emit, wtile, wcols):
            import os as _osA
            with tc.tile_pool(name="a_sb", bufs=int(_osA.environ.get("ASB", "4"))) as sbp, \
                 tc.tile_pool(name="a_ps", bufs=int(_osA.environ.get("APS", "4")), space="PSUM") as psp, \
                 tc.tile_pool(name="a_st", bufs=int(_osA.environ.get("AST", "3")), space="PSUM") as stp:
                for t0 in range(0, ntiles, ATILE):
                    bt = min(ATILE, ntiles - t0)
                    hT4 = sbp.tile([P, ATILE, 2, P], bf16, tag="hT4")
                    nc.sync.dma_start(
                        out=hT4[:, 0:bt, :, :],
                        in_=srcT_dram[t0:t0 + bt].rearrange("t p b e -> p t b e"))
                    sq4 = sbp.tile([P, ATILE, 2, P], bf16, tag="sq4")
                    nc.scalar.activation(out=sq4[:, 0:bt], in_=hT4[:, 0:bt],
                                         func=AF.Square)
                    sps = stp.tile([P, ATILE, 2], f32, tag="sps")
                    for i in range(bt):
                        for b in range(2):
                            nc.tensor.matmul(out=sps[:, i, 0:1],
                                             lhsT=hT4[:, i, b, :], rhs=ones_c[:],
                                             start=(b == 0), stop=(b == 1))
                        for b in range(2):
                            nc.tensor.matmul(out=sps[:, i, 1:2],
                                             lhsT=sq4[:, i, b, :], rhs=ones_c[:],
                                             start=(b == 0), stop=(b == 1))
                    # var = sumsq/256 - (sum/256)^2 ; rstd = 1/sqrt(var+eps)
                    mean4 = sbp.tile([P, ATILE], f32, tag="mean4")
                    nc.vector.tensor_scalar(out=mean4[:, 0:bt],
                                            in0=sps[:, 0:bt, 0],
                                            scalar1=1.0 / IN_FEATS, scalar2=None,
                                            op0=Alu.mult)
                    mu2 = sbp.tile([P, ATILE], f32, tag="mu2")
                    nc.scalar.activation(out=mu2[:, 0:bt], in_=sps[:, 0:bt, 0],
                                         func=AF.Square, scale=1.0 / IN_FEATS)
                    var4 = sbp.tile([P, ATILE], f32, tag="var4")
                    nc.vector.tensor_scalar(out=var4[:, 0:bt],
                                            in0=sps[:, 0:bt, 1],
                                            scalar1=1.0 / IN_FEATS,
                                            scalar2=None, op0=Alu.mult)
                    vc4 = sbp.tile([P, ATILE], f32, tag="vc4")
                    nc.vector.tensor_tensor(out=vc4[:, 0:bt], in0=var4[:, 0:bt],
                                            in1=mu2[:, 0:bt], op=Alu.subtract)
                    sd4 = sbp.tile([P, ATILE], f32, tag="sd4")
                    nc.scalar.activation(out=sd4[:, 0:bt], in_=vc4[:, 0:bt],
                                         func=AF.Sqrt, bias=eps_c[:])
                    rstd4 = sbp.tile([P, ATILE], f32, tag="rstd4")
                    nc.vector.reciprocal(out=rstd4[:, 0:bt], in_=sd4[:, 0:bt])
                    batch = []
                    for i in range(bt):
                        g = psp.tile([P, wcols], f32, tag="gemm")
                        nc.tensor.matmul(out=g[:], lhsT=hT4[:, i, 0, :],
                                         rhs=wtile[:, 0, :], start=True, stop=False)
                        nc.tensor.matmul(out=g[:], lhsT=hT4[:, i, 1, :],
                                         rhs=wtile[:, 1, :], start=False, stop=True)
                        emit(t0 + i, i, g, mean4, rstd4, sbp, batch)
                    if batch:
                        st4, bt0 = batch[0]
                        nc.sync.dma_start(
                            out=fsv_t[bt0 * P:(bt0 + bt) * P, :].rearrange(
                                "(t p) f -> p t f", p=P),
                            in_=st4[:, 0:bt, :])

        def emit_fsv(t, i, g, mean4, rstd4, sbp, batch):
            if i == 0:
                st4 = sbp.tile([P, ATILE, TBL_COLS], bf16, tag="fsv4")
                batch.append((st4, t))
            st4, _ = batch[0]
            nc.vector.tensor_scalar(
                out=st4[:, i, 0:OUT_FEATS], in0=g[:, 0:OUT_FEATS],
                scalar1=rstd4[:, i:i + 1], scalar2=None, op0=Alu.mult)
            nc.scalar.activation(out=st4[:, i, OUT_FEATS:], in_=g[:, OUT_FEATS:],
                                 func=AF.Copy, scale=rstd4[:, i:i + 1])

        def emit_fd(t, i, g, mean4, rstd4, sbp, batch):
            nc.vector.tensor_scalar(
                out=fd_slice[:, t, 0:P], in0=g[:, 0:P],
                scalar1=rstd4[:, i:i + 1], scalar2=None, op0=Alu.mult)
            nc.scalar.activation(out=fd_slice[:, t, P:OUT_FEATS], in_=g[:, P:],
                                 func=AF.Copy, scale=rstd4[:, i:i + 1])
            nc.vector.tensor_copy(out=stats_my[:, t, 0:1], in_=mean4[:, i:i + 1])
            nc.vector.tensor_copy(out=stats_my[:, t, 1:2],
                                  in_=rstd4[:, i:i + 1])

        node_sweep(featmyT_t, NWIN, emit_fd, wfd, OUT_FEATS)
        if "A" in phases:
            node_sweep(featTb_t, NTILES, emit_fsv, wfsv, TBL_COLS)

        # ---------------- Phase B: edge phase ----------------
        fsv_hi = fsv_t[HALF:N_PAD, :]
        if "B" not in phases:
            NWIN_B = 0
        else:
            NWIN_B = NWIN
        import os as _os
        _bb = lambda k, d: int(_os.environ.get(k, d))
        with tc.tile_pool(name="b_glo", bufs=_bb("BGL", 4)) as glop, \
             tc.tile_pool(name="b_ghi", bufs=_bb("BGH", 4)) as ghip, \
             tc.tile_pool(name="b_ohg", bufs=_bb("BOG", 3)) as ohgp, \
             tc.tile_pool(name="b_oha", bufs=_bb("BOA", 3)) as ohap, \
             tc.tile_pool(name="b_sev", bufs=_bb("BSV", 4)) as sevp, \
             tc.tile_pool(name="b_e8", bufs=_bb("BE8", 2)) as e8p, \
             tc.tile_pool(name="b_md", bufs=_bb("BMD", 3)) as mdp, \
             tc.tile_pool(name="b_hw", bufs=_bb("BHW", 3)) as hwp, \
             tc.tile_pool(name="b_ev", bufs=_bb("BEV", 2), space="PSUM") as evp, \
             tc.tile_pool(name="b_sc", bufs=_bb("BSC", 2), space="PSUM") as scp, \
             tc.tile_pool(name="b_hu", bufs=_bb("BHU", 2), space="PSUM") as hup:

            icol_of = np.concatenate([[0], np.cumsum([8 * c for c in cpw])])
            _qctr = [0]

            def _next_q():
                q = _qctr[0] % GQN
                _qctr[0] += 1
                return q

            def win_gather(w):
                """Issue gathers + oh loads + one-hot builds for window w."""
                cb = cbase[w]
                icol = int(icol_of[w])
                tiles = {}
                ohg_w = ohgp.tile([P, CPWMX, P], f8, tag="ohg")
                if LVL >= 2:
                    nc.sync.dma_start(out=ohg_w[:, 0:cpw[w], :],
                                      in_=ohg_t[:, cb:cb + cpw[w], :])
                import os as _os2
                _SEQDMA = _os2.environ.get("DBG_SEQDMA") == "1"
                # >1024 idx per dma_gather call wedges the device
                MAXC = int(_os2.environ.get("MAXC", "8"))
                _SPK = _os2.environ.get("GSP", "1") == "1"
                if lowC[w] and _SEQDMA:
                    G = glop.tile([P, LCMX, TBL_COLS], bf16, tag="Glo")
                    nc.sync.dma_start(
                        out=G[:, 0:lowC[w], :],
                        in_=fsv_t[0:lowC[w] * P, :].rearrange(
                            "(t p) f -> p t f", p=P))
                    tiles["lo"] = G
                    icol += 8 * lowC[w]
                if highC[w] and _SEQDMA:
                    G = ghip.tile([P, HCMX, TBL_COLS], bf16, tag="Ghi")
                    nc.sync.dma_start(
                        out=G[:, 0:highC[w], :],
                        in_=fsv_t[0:highC[w] * P, :].rearrange(
                            "(t p) f -> p t f", p=P))
                    tiles["hi"] = G
                    icol += 8 * highC[w]
                if lowC[w] and not _SEQDMA:
                    G = glop.tile([P, LCMX, TBL_COLS], bf16, tag="Glo")
                    o = 0
                    while o < (lowC[w] if LVL >= 1 else 0):
                        g = min(MAXC, lowC[w] - o)
                        ni = g * P
                        nc.gpsimd.dma_gather(
                            out_ap=G[:, o:o + g, :], in_ap=fsv_t[:, :],
                            idxs_ap=gidx[:, icol:icol + ni // 16],
                            num_idxs=ni, num_idxs_reg=ni, elem_size=TBL_COLS,
                            single_packet=_SPK, queue_num=_next_q())
                        icol += ni // 16
                        o += g
                    tiles["lo"] = G
                if highC[w] and not _SEQDMA:
                    G = ghip.tile([P, HCMX, TBL_COLS], bf16, tag="Ghi")
                    o = 0
                    while o < (highC[w] if LVL >= 1 else 0):
                        g = min(MAXC, highC[w] - o)
                        ni = g * P
                        nc.gpsimd.dma_gather(
                            out_ap=G[:, o:o + g, :], in_ap=fsv_hi,
                            idxs_ap=gidx[:, icol:icol + ni // 16],
                            num_idxs=ni, num_idxs_reg=ni, elem_size=TBL_COLS,
                            single_packet=_SPK, queue_num=_next_q())
                        icol += ni // 16
                        o += g
                    tiles["hi"] = G
                oha_w = ohap.tile([P, CPWMX, P], bf16, tag="oha")
                if LVL >= 2:
                    nc.vector.tensor_tensor(
                        out=oha_w[:, 0:cpw[w], :],
                        in0=iotab[:, None, :].to_broadcast([P, cpw[w], P]),
                        in1=dstf[:, cb:cb + cpw[w], None].to_broadcast(
                            [P, cpw[w], P]),
                        op=Alu.is_equal)
                return tiles, ohg_w, oha_w

            def chunk_of(w, c):
                """(section G-key, local idx) for chunk c of window w."""
                if c < lowC[w]:
                    return "lo", c
                return "hi", c - lowC[w]

            SG = int(_os.environ.get("BSG", "4"))  # chunks per silu group
            NPAR = int(_os.environ.get("BNP", "3"))  # windows in flight
            assert NPAR * CPWMX * NUM_HEADS * 4 <= 2048, "score tile > PSUM bank"

            def score_group(w, wi, tiles, ohg_w, score_ps, c0):
                gn = min(SG, cpw[w] - c0)
                ev2 = evp.tile([P, 2, SG, P], f32, tag="ev2")
                if LVL >= 3:
                    for i in range(gn):
                        sec, lc = chunk_of(w, c0 + i)
                        G = tiles[sec]
                        for b in range(2):
                            nc.tensor.matmul(
                                out=ev2[:, b, i, :],
                                lhsT=fd_slice[:, w, b * P:(b + 1) * P],
                                rhs=ohg_w[:, c0 + i, :], start=True, stop=False)
                            nc.tensor.matmul(
                                out=ev2[:, b, i, :],
                                lhsT=G[:, lc, b * P:(b + 1) * P],
                                rhs=identb[:], start=False, stop=True)
                sevT = sevp.tile([P, 2, SG, P], bf16, tag="sevT")
                if LVL >= 4:
                    nc.scalar.activation(out=sevT[:, :, 0:gn, :],
                                         in_=ev2[:, :, 0:gn, :], func=AF.Silu)
                for i in range(gn if LVL >= 5 else 0):
                    for b in range(2):
                        nc.tensor.matmul(
                            out=score_ps[:, wi, c0 + i, :],
                            lhsT=sevT[:, b, i, :], rhs=attnb[:, b, :],
                            start=(b == 0), stop=(b == 1))

            def win_finish(w, wi, tiles, ohg_w, oha_w, e8g):
                # md[:, c, 0:256] = fv * exp(score) broadcast; md[:, c, 256:264]
                # = exp(score) so ONE agg matmul also produces denominators.
                md = mdp.tile([P, CPWMX, AGG_COLS], bf16, tag="md")
                if LVL >= 5:
                    nc.vector.tensor_copy(out=md[:, 0:cpw[w], OUT_FEATS:],
                                          in_=e8g[:, wi, 0:cpw[w], :])
                for sec, g, off in (("lo", lowC[w], 0), ("hi", highC[w], lowC[w])):
                    if not g or LVL < 6:
                        continue
                    G = tiles[sec]
                    nc.vector.tensor_tensor(
                        out=md[:, off:off + g, 0:OUT_FEATS].rearrange(
                            "p c (f h) -> p c f h", h=NUM_HEADS),
                        in0=G[:, 0:g, OUT_FEATS:].rearrange(
                            "p c (f h) -> p c f h", h=NUM_HEADS),
                        in1=md[:, off:off + g, None, OUT_FEATS:].to_broadcast(
                            [P, g, FPH, NUM_HEADS]),
                        op=Alu.mult)
                # note: matmul accumulation groups must stay contiguous per
                # PSUM region on HW - interleaving two regions corrupts one.
                if LVL < 7:
                    return
                hupd = hup.tile([P, AGG_COLS], f32, tag="hupd")
                for c in range(cpw[w]):
                    nc.tensor.matmul(out=hupd[:, 0:AGG_COLS],
                                     lhsT=oha_w[:, c, :], rhs=md[:, c, :],
                                     start=(c == 0), stop=(c == cpw[w] - 1))
                nc.vector.tensor_copy(out=husum[:, w, :], in_=hupd[:])

            LVL = int(_os.environ.get("DBG_LEVEL", "99"))
            w = 0
            while w < NWIN_B:
                grp = list(range(w, min(w + NPAR, NWIN_B)))
                nw = len(grp)
                score_ps = scp.tile([P, NPAR, CPWMX, NUM_HEADS], f32,
                                    tag="score")
                state = []
                for wi, ww in enumerate(grp):
                    tiles, ohg_w, oha_w = win_gather(ww)
                    state.append((ww, wi, tiles, ohg_w, oha_w))
                maxg = max((cpw[ww] + SG - 1) // SG for ww in grp)
                for gi in range(maxg):
                    for (ww, wi, tiles, ohg_w, oha_w) in state:
                        if gi * SG < cpw[ww]:
                            score_group(ww, wi, tiles, ohg_w, score_ps, gi * SG)
                # one exp per window-group: silu<->exp share no ACT table set,
                # so each transition costs a 1.3us table load - batch them.
                e8g = e8p.tile([P, NPAR, CPWMX, NUM_HEADS], bf16, tag="e8g")
                if LVL >= 5:
                    nc.scalar.activation(out=e8g[:, 0:nw], in_=score_ps[:, 0:nw],
                                         func=AF.Exp)
                for (ww, wi, tiles, ohg_w, oha_w) in state:
                    win_finish(ww, wi, tiles, ohg_w, oha_w, e8g)
                w += len(grp)

        # ---------------- Phase C: normalize + residual + silu ----------------
        with tc.tile_pool(name="c_sb", bufs=3) as sbp:
            for w0 in range(0, NWIN if "C" in phases else 0, ATILE):
                bt = min(ATILE, NWIN - w0)
                hu4 = husum[:, w0:w0 + bt, :]
                F4 = sbp.tile([P, ATILE, IN_FEATS], bf16, tag="F4")
                nc.sync.dma_start(
                    out=F4[:, 0:bt, :],
                    in_=featmy_t[w0 * P:(w0 + bt) * P, :].rearrange(
                        "(t p) f -> p t f", p=P))
                h4 = sbp.tile([P, ATILE, IN_FEATS], bf16, tag="h")
                for i in range(bt):
                    w = w0 + i
                    nc.vector.tensor_scalar(
                        out=h4[:, i, :], in0=F4[:, i, :],
                        scalar1=stats_my[:, w, 0:1],
                        scalar2=stats_my[:, w, 1:2],
                        op0=Alu.subtract, op1=Alu.mult)
                den = sbp.tile([P, ATILE, NUM_HEADS], f32, tag="den")
                nc.vector.tensor_scalar_add(out=den[:, 0:bt],
                                            in0=hu4[:, 0:bt, OUT_FEATS:],
                                            scalar1=1e-30)
                denr = sbp.tile([P, ATILE, NUM_HEADS], f32, tag="denr")
                nc.vector.reciprocal(out=denr[:, 0:bt], in_=den[:, 0:bt])
                o = sbp.tile([P, ATILE, FPH, NUM_HEADS], f32, tag="o")
                nc.vector.tensor_tensor(
                    out=o[:, 0:bt],
                    in0=hu4[:, 0:bt, 0:OUT_FEATS].rearrange(
                        "p w (f h) -> p w f h", h=NUM_HEADS),
                    in1=denr[:, 0:bt, None, :].to_broadcast(
                        [P, bt, FPH, NUM_HEADS]),
                    op=Alu.mult)
                on = sbp.tile([P, ATILE, OUT_FEATS], f32, tag="on")
                nc.vector.tensor_tensor(
                    out=on[:, 0:bt].rearrange("p w (h f) -> p w h f",
                                              h=NUM_HEADS),
                    in0=o[:, 0:bt].rearrange("p w f h -> p w h f"),
                    in1=h4[:, 0:bt].rearrange("p w (h f) -> p w h f",
                                              h=NUM_HEADS),
                    op=Alu.add)
                oo = sbp.tile([P, ATILE, OUT_FEATS], bf16, tag="oo")
                nc.scalar.activation(out=oo[:, 0:bt], in_=on[:, 0:bt],
                                     func=AF.Silu)
                nc.sync.dma_start(
                    out=out_t[w0 * P:(w0 + bt) * P, :].rearrange(
                        "(w p) f -> p w f", p=P),
                    in_=oo[:, 0:bt])

    nc.compile()
    return nc, (lowC, highC)


def _derive_schedule(src, dst):
    """Per-window chunk counts (low/high table half), maxed over cores."""
    lowC = np.zeros(NWIN, np.int64)
    highC = np.zeros(NWIN, np.int64)
    for core in range(N_CORES):
        lo, hi = core * SLICE, (core + 1) * SLICE
        m = (dst >= lo) & (dst < hi)
        w_of = (dst[m] - lo) // P
        is_lo = src[m] < HALF
        cl = np.bincount(w_of[is_lo], minlength=NWIN)
        ch = np.bincount(w_of[~is_lo], minlength=NWIN)
        lowC = np.maximum(lowC, (cl + P - 1) // P)
        highC = np.maximum(highC, (ch + P - 1) // P)
    return tuple(int(x) for x in lowC), tuple(int(x) for x in highC)


def _prepare_core_inputs(core, src, dst, lowC, highC):
    import ml_dtypes
    cpw = [l + h for l, h in zip(lowC, highC)]
    C_TOT = sum(cpw)
    cbase = np.concatenate([[0], np.cumsum(cpw)]).astype(int)

    lo, hi = core * SLICE, (core + 1) * SLICE
    m = (dst >= lo) & (dst < hi)
    dsl = dst[m] - lo
    ssl = src[m]
    w_of = dsl // P
    is_lo = ssl < HALF

    slot_src = np.zeros((C_TOT, P), np.int64)
    slot_doff = np.full((C_TOT, P), -1.0, np.float32)

    # sort each (window, table-half) section by src so the gather reads the
    # table in ascending row order (HBM locality)
    order = np.lexsort((ssl, ~is_lo, w_of))
    ssl_o, dsl_o, w_o, lo_o = ssl[order], dsl[order], w_of[order], is_lo[order]
    for w in range(NWIN):
        wm = w_o == w
        for half, cb, g in ((True, cbase[w], lowC[w]),
                            (False, cbase[w] + lowC[w], highC[w])):
            sel = wm & (lo_o == half)
            s_w = ssl_o[sel]
            d_w = dsl_o[sel] - w * P
            n = len(s_w)
            assert n <= g * P, (core, w, half, n, g * P)
            flat_s = slot_src[cb:cb + g].reshape(-1)
            flat_d = slot_doff[cb:cb + g].reshape(-1)
            flat_s[:n] = s_w
            flat_d[:n] = d_w

    dstf = slot_doff.T.copy().astype(ml_dtypes.bfloat16)  # [P, C_TOT]

    ohg = np.zeros((P, C_TOT, P), ml_dtypes.float8_e4m3)
    cc, ee = np.nonzero(slot_doff >= 0)
    ohg[slot_doff[cc, ee].astype(np.int64), cc, ee] = 1

    idx_cols = []
    for w in range(NWIN):
        for half, cb, g in ((True, cbase[w], lowC[w]),
                            (False, cbase[w] + lowC[w], highC[w])):
            if not g:
                continue
            s = slot_src[cb:cb + g].reshape(-1).copy()
            if not half:
                s = np.maximum(s - HALF, 0)
            idx = s.astype(np.int16)
            idx_cols.append(np.tile(idx.reshape(-1, 16).T, (8, 1)))
    gidx = np.concatenate(idx_cols, axis=1).astype(np.int16)
    return dstf, gidx, ohg


def _shared_inputs(inputs):
    import ml_dtypes
    feat = np.asarray(inputs["feat"], np.float32)
    Wsrc = np.asarray(inputs["Wsrc"], np.float32)
    Wdst = np.asarray(inputs["Wdst"], np.float32)
    Wval = np.asarray(inputs["Wval"], np.float32)
    attn = np.asarray(inputs["attn"], np.float32).reshape(NUM_HEADS, FPH)

    featb = np.zeros((N_PAD, IN_FEATS), ml_dtypes.bfloat16)
    featb[:N_NODES] = feat.astype(ml_dtypes.bfloat16)
    # pre-transposed/tiled: [tile, f-partition, block, node]
    featTb = np.ascontiguousarray(
        featb.reshape(NTILES, P, 2, P).transpose(0, 3, 2, 1))

    # weights transposed + output-column permuted to (f,h)-major.
    # LayerNorm mean-fold: h@W^T = rstd*(feat@W'^T) with W' = W - rowmean(W)
    # (the -mu*ones part of LN folds into centered weight rows; rstd is
    # applied per-node after the GEMM).
    WsrcP = Wsrc[_OLD_OF_NEW, :]   # [256 newcol, 256 in]
    WvalP = Wval[_OLD_OF_NEW, :]
    WdstP = Wdst[_OLD_OF_NEW, :]
    WsrcP = WsrcP - WsrcP.mean(axis=1, keepdims=True)
    WvalP = WvalP - WvalP.mean(axis=1, keepdims=True)
    WdstP = WdstP - WdstP.mean(axis=1, keepdims=True)
    wfsv = np.zeros((P, 2, TBL_COLS), np.float32)
    for b in range(2):
        wfsv[:, b, 0:OUT_FEATS] = WsrcP[:, b * P:(b + 1) * P].T
        wfsv[:, b, OUT_FEATS:] = WvalP[:, b * P:(b + 1) * P].T
    wfd = np.zeros((P, 2, OUT_FEATS), np.float32)
    for b in range(2):
        wfd[:, b, :] = WdstP[:, b * P:(b + 1) * P].T

    attnb = np.zeros((P, 2, NUM_HEADS), np.float32)
    for b in range(2):
        j = b * P + np.arange(P)
        attnb[np.arange(P), b, j % NUM_HEADS] = attn[j % NUM_HEADS,
                                                     j // NUM_HEADS]

    identb = np.eye(P, dtype=ml_dtypes.bfloat16)
    iotab = np.tile(np.arange(P, dtype=ml_dtypes.bfloat16).reshape(1, P),
                    (P, 1))
    bf = ml_dtypes.bfloat16
    return (featTb, feat, wfsv.astype(bf), wfd.astype(bf), attnb.astype(bf),
            identb, iotab)


def make_in_maps(inputs, lowC, highC):
    import ml_dtypes
    featTb, feat, wfsv, wfd, attnb, identb, iotab = _shared_inputs(inputs)
    src = np.asarray(inputs["src"], np.int64)
    dst = np.asarray(inputs["dst"], np.int64)
    in_maps = []
    for core in range(N_CORES):
        dstf, gidx, ohg = _prepare_core_inputs(core, src, dst, lowC, highC)
        featmy = np.zeros((SLICE_PAD, IN_FEATS), ml_dtypes.bfloat16)
        n = min(SLICE_PAD, N_NODES - core * SLICE)
        featmy[:n] = feat[core * SLICE:core * SLICE + n].astype(
            ml_dtypes.bfloat16)
        featmyT = np.ascontiguousarray(
            featmy.reshape(NWIN, P, 2, P).transpose(0, 3, 2, 1))
        in_maps.append(dict(
            featTb=featTb, featmyT=featmyT, featmy=featmy, wfsv=wfsv, wfd=wfd,
            attnb=attnb, identb=identb, iotab=iotab, dstf=dstf, gidx=gidx,
            ohg=ohg,
        ))
    return in_maps


def kernel(**inputs):
    import concourse.bass_utils as bass_utils

    for b in ("bsrc", "bdst", "bval"):
        assert not np.any(np.asarray(inputs[b])), \
            "nonzero biases unsupported by this kernel"
    src = np.asarray(inputs["src"], np.int64)
    dst = np.asarray(inputs["dst"], np.int64)

    lowC, highC = _derive_schedule(src, dst)
    key = (lowC, highC)
    if key not in _CACHE:
        _CACHE[key] = _build_nc(lowC, highC)
    nc, _ = _CACHE[key]

    in_maps = make_in_maps(inputs, lowC, highC)
    res = bass_utils.run_bass_kernel_spmd(nc, in_maps, core_ids=list(range(N_CORES)))
    out = np.concatenate(
        [res.results[c]["outmy"][:SLICE] for c in range(N_CORES)], axis=0)
    return np.ascontiguousarray(out.astype(np.float32))

